# revision 1
# baseline (speedup 1.0000x reference)
"""Trainium2 Bass kernel for nn_MixedAttnHeadEmbed (mixed-head-config attention).

Math (per batch b):
  Two attention configs share q_m/k_m/v_m [B,T,2048]:
    A: h=8  heads, d_max=256, mixing e in {1024,2048} -> d in {128,256}, weights w0,w1
    B: h=16 heads, d_max=128, mixing e in {1024,2048} -> d in {64,128},  weights w2,w3
  Each config: per-head q/k slices are RoPE'd, weight-summed (padded to d_max),
  GQA (8 kv heads), causal softmax attention; outputs of both configs sum.

Sharding: 8 cores = 4 batches x 2 shards. Shard s owns A-heads [4s,4s+4) and
B-heads [8s,8s+8) -> both write output columns [1024s, 1024s+1024) which are
summed on device; per-core output is the transposed block outT [1024, T].

Device layout: scores computed transposed (sT[k,q], k on partitions) so the
softmax'd weights feed the y^T matmul with no on-chip transposes; softmax is
max-free (scores are provably < 2 for this problem family; exp is safe in
fp32) with the denominator from an all-ones stationary matmul.
"""

import math
from contextlib import ExitStack
from dataclasses import dataclass

import numpy as np

import concourse.bass as bass
import concourse.mybir as mybir
import concourse.tile as tile
from concourse import bacc

F32 = mybir.dt.float32
F32R = mybir.dt.float32r
NEG = -1e9
P = 128


@dataclass(frozen=True)
class KCfg:
    T: int = 1024       # sequence length
    NA: int = 4         # config-A heads per core (d_max=256)
    NB: int = 8         # config-B heads per core (d_max=128); must be 2*NA
    REG: int = 512      # psum region width (<=512)

    @property
    def TK(self):
        return self.T // P

    @property
    def NREG(self):
        return self.T // self.REG

    @property
    def NKVB(self):
        return self.NB // 2

    @property
    def ROWS(self):
        return self.NA * 256  # == NB * 128 output rows per core


FULL = KCfg()


def _in_specs(cfg: KCfg):
    T = cfg.T
    return {
        "qT1": (cfg.NA * 128, T),
        "qT2": (cfg.NA * 256, T),
        "kTa1": (cfg.NA * 128, T),
        "kTa2": (cfg.NA * 256, T),
        "kTb1": (cfg.NKVB * 64, T),
        "kTb2": (cfg.NKVB * 128, T),
        "va1": (T, cfg.NA * 128),
        "va2": (T, cfg.NA * 256),
        "vb1": (T, cfg.NKVB * 64),
        "vb2": (T, cfg.NKVB * 128),
        "ca1": (128, T), "sa1": (128, T),
        "ca2": (256, T), "sa2": (256, T),
        "cb1": (128, T), "sb1": (128, T),
        "cb2": (128, T), "sb2": (128, T),
        "wvec": (P, 4),
    }


class _EngPick:
    """Static load balancer across DVE / GPSIMD / ACT.

    units: 1.0 ~ one [.,1024] fp32 pass. Cost-model calibration: DVE and
    Pool run TT at ~1 elem/lane/cycle (fp32 has no DVE fast mode); ACT can
    only take single-input copies, and it also carries all the exps (those
    are tallied in via act())."""

    GP_W = 1.05   # tuned: bias work toward pool
    ACT_W = 1.5

    def __init__(self, nc):
        self.nc = nc
        self.load = {"dve": 0.0, "pool": 0.0, "act": 0.0}

    def dve(self, units=1.0):
        self.load["dve"] += units
        return self.nc.vector

    def act(self, units=1.0):
        self.load["act"] += units * self.ACT_W
        return self.nc.scalar

    def tt(self, units=1.0):
        """2-input sbuf op: DVE or GPSIMD."""
        if self.load["dve"] + units <= self.load["pool"] + self.GP_W * units:
            return self.dve(units)
        self.load["pool"] += self.GP_W * units
        return self.nc.gpsimd

    def copy(self, dst, src, units=1.0):
        """1-input copy: any of the three engines."""
        costs = {"dve": units, "pool": self.GP_W * units,
                 "act": self.ACT_W * units}
        eng = min(costs, key=lambda k: self.load[k] + costs[k])
        self.load[eng] += costs[eng]
        if eng == "act":
            self.nc.scalar.copy(dst, src)
        elif eng == "pool":
            self.nc.gpsimd.tensor_copy(dst, src)
        else:
            self.nc.vector.tensor_copy(dst, src)


def build_program(cfg: KCfg = FULL):
    # Bacc (not plain Bass): its compile() runs generate_event_semaphores,
    # which splits multi-wait sync_infos — TRN2 allows 1 wait per instruction.
    nc = bacc.Bacc("TRN2", target_bir_lowering=False)
    T, TK, REG, NREG = cfg.T, cfg.TK, cfg.REG, cfg.NREG
    RPB = REG // P  # k-chunks per region

    D = {}
    for name, shape in _in_specs(cfg).items():
        D[name] = nc.declare_dram_parameter(name, list(shape), F32, isOutput=False)
    outT = nc.declare_dram_parameter("outT", [cfg.ROWS, T], F32, isOutput=True)
    RB = cfg.ROWS // P

    mult, add = mybir.AluOpType.mult, mybir.AluOpType.add

    with ExitStack() as ctx:
        tc = ctx.enter_context(tile.TileContext(nc))
        const = ctx.enter_context(tc.tile_pool(name="const", bufs=1))
        rawp = ctx.enter_context(tc.tile_pool(name="raw", bufs=2))
        mixp = ctx.enter_context(tc.tile_pool(name="mix", bufs=2))
        scr = ctx.enter_context(tc.tile_pool(name="scr", bufs=1))
        ppool = ctx.enter_context(tc.tile_pool(name="pp", bufs=3))
        accp = ctx.enter_context(tc.tile_pool(name="acc", bufs=1))
        normp = ctx.enter_context(tc.tile_pool(name="norm", bufs=1))
        spsum = ctx.enter_context(tc.tile_pool(name="spsum", bufs=2, space="PSUM"))
        ypsum = ctx.enter_context(tc.tile_pool(name="ypsum", bufs=1, space="PSUM"))
        dpsum = ctx.enter_context(tc.tile_pool(name="dpsum", bufs=1, space="PSUM"))

        pick = _EngPick(nc)

        # ---- constants ----
        ones_f = const.tile([P, P], F32, name="ones_f")
        nc.vector.memset(ones_f, 1.0)
        ones = const.tile([P, P], F32R)
        nc.vector.tensor_copy(ones, ones_f)  # rounds to f32r for the matmul
        dmask = const.tile([P, P], F32)
        nc.gpsimd.memset(dmask, 0.0)
        # dmask[k, q] = 0 where q >= k else NEG  (transposed causal diag block)
        nc.gpsimd.affine_select(
            out=dmask, in_=dmask, compare_op=mybir.AluOpType.is_ge,
            fill=NEG, base=0, pattern=[[1, P]], channel_multiplier=-1,
        )
        tabs = {}
        for nm in ("ca1", "sa1", "ca2", "sa2", "cb1", "sb1", "cb2", "sb2"):
            rows = _in_specs(cfg)[nm][0]
            tl = const.tile([P, rows // P, T], F32, name=nm, tag=nm)
            tabs[nm] = tl
            nc.sync.dma_start(out=tl, in_=D[nm].rearrange("(c p) t -> p c t", p=P))
        wv = const.tile([P, 4], F32)
        nc.sync.dma_start(out=wv, in_=D["wvec"][:, :])

        outacc = accp.tile([P, RB, T], F32)

        def halfmul(dst, src, tab, half, base=0, rows=P):
            """dst[base:base+rows][j] = src[sigma(j)] * tab_math[j], where
            sigma swaps halves of size `half` within each 2*half group.

            tab is the HOST-SIGMA-PERMUTED signed sin table, so the multiply
            is same-base (u = src*tab) and the rotation becomes 1-input
            cross-base copies (the only cross-partition-base op trn2 allows).
            """
            u = scr.tile([P, T], F32, tag="xbt", name="xbt")
            usl = u[base:base + rows, :]
            pick.tt(1.0).tensor_tensor(usl, src, tab, mult)
            for g in range(rows // (2 * half)):
                b0 = base + 2 * half * g
                pick.copy(dst[b0:b0 + half, :], u[b0 + half:b0 + 2 * half, :], 1.0)
                pick.copy(dst[b0 + half:b0 + 2 * half, :], u[b0:b0 + half, :], 1.0)

        def xb_add(dst, src, units):
            """dst += src across partition bases (copy to re-base, then add)."""
            n = src.shape[0]
            tmp = scr.tile([P, T], F32, tag="xbt2", name="xbt2")
            view = tmp[0:n, :]
            pick.copy(view, src, units)
            pick.tt(units).tensor_tensor(dst, dst, view, add)

        def mix_qk_A(out, x1, x2, c1, s1, c2, s2):
            """out [P,2,T] = rope-mix for a config-A head.
            x1 [P,T] (d=128 slice), x2 [P,2,T] (d=256 slice).
            s1 is sigma64-permuted; s2 is the plain signed sin table."""
            t1 = scr.tile([P, T], F32, tag="t1")
            t2 = scr.tile([P, T], F32, tag="t2")
            # dc0: x2t0*c2_0 + x2t1*s2_0 + x1*c1 + shift64(x1)*s1
            pick.tt().tensor_tensor(out[:, 0, :], x2[:, 0, :], c2[:, 0, :], mult)
            pick.tt().tensor_tensor(t1, x2[:, 1, :], s2[:, 0, :], mult)
            pick.tt().tensor_tensor(out[:, 0, :], out[:, 0, :], t1, add)
            pick.tt().tensor_tensor(t1, x1, c1[:, 0, :], mult)
            halfmul(t2, x1, s1[:, 0, :], 64)
            pick.tt().tensor_tensor(t1, t1, t2, add)
            pick.tt().tensor_tensor(out[:, 0, :], out[:, 0, :], t1, add)
            # dc1: x2t1*c2_1 + x2t0*s2_1
            pick.tt().tensor_tensor(out[:, 1, :], x2[:, 1, :], c2[:, 1, :], mult)
            pick.tt().tensor_tensor(t1, x2[:, 0, :], s2[:, 1, :], mult)
            pick.tt().tensor_tensor(out[:, 1, :], out[:, 1, :], t1, add)

        def mix_qk_B_pair(out, x1p, x2p, c1, s1, c2, s2):
            """out [P,2,T]: B-head pair. out[:,j,:] for heads (2p+j).
            x2p [P,2,T] (d=128 per head), x1p [P,T] packed pair (d=64 each).
            s2 sigma64-permuted; s1 sigma32-permuted."""
            t1 = scr.tile([P, T], F32, tag="t1")
            t2 = scr.tile([P, T], F32, tag="t2")
            for j in range(2):
                pick.tt().tensor_tensor(out[:, j, :], x2p[:, j, :], c2[:, 0, :], mult)
                halfmul(t1, x2p[:, j, :], s2[:, 0, :], 64)
                pick.tt().tensor_tensor(out[:, j, :], out[:, j, :], t1, add)
            # packed d=64 contributions for both heads of the pair
            pick.tt().tensor_tensor(t1, x1p, c1[:, 0, :], mult)
            halfmul(t2, x1p, s1[:, 0, :], 32)
            pick.tt().tensor_tensor(t1, t1, t2, add)
            pick.tt(1.0).tensor_tensor(out[0:64, 0, :], out[0:64, 0, :],
                                       t1[0:64, :], add)
            xb_add(out[0:64, 1, :], t1[64:128, :], 1.0)

        def subchunks(c):
            out = []
            for r in range(NREG):
                q0 = max(REG * r, P * c)
                q1 = REG * (r + 1)
                if q1 > q0:
                    out.append((r, q0, q1 - q0))
            return out

        def attn_head(qmixs, kmixs, vmix, blks, is_b):
            """qmixs/kmixs: per-d-chunk [P, T] APs; vmix [P, TK, ndc*P].

            Matmul operands are bitcast to float32r: full-rate PE streaming
            (fp32 proper runs at 1/4 rate) with near-fp32 accumulation."""
            ndc = len(qmixs)
            den = dpsum.tile([P, T], F32, tag="den")
            yts = [ypsum.tile([P, T], F32, tag=f"yt{i}", name=f"yt{i}")
                   for i in range(ndc)]
            for c in range(TK):
                for (r, q0, n) in subchunks(c):
                    last_c = min(TK, RPB * (r + 1)) - 1
                    sT = spsum.tile([P, REG], F32, tag="sT")
                    for dc in range(ndc):
                        nc.tensor.matmul(
                            sT[:, :n],
                            kmixs[dc][:, P * c:P * (c + 1)],
                            qmixs[dc][:, q0:q0 + n],
                            start=(dc == 0), stop=(dc == ndc - 1))
                    if q0 == P * c:  # diagonal block gets the causal mask
                        pick.dve(0.125).tensor_tensor(sT[:, :P], sT[:, :P],
                                                      dmask, add)
                    pt = ppool.tile([P, REG], F32R, tag="pT")
                    pick.act(n / 1024.0).activation(
                        pt[:, :n], sT[:, :n], mybir.ActivationFunctionType.Exp)
                    for dc in range(ndc):
                        nc.tensor.matmul(
                            yts[dc][:, q0:q0 + n],
                            vmix[:, c, P * dc:P * (dc + 1)],
                            pt[:, :n],
                            start=(c == 0), stop=(c == last_c))
                    nc.tensor.matmul(den[:, q0:q0 + n], ones,
                                     pt[:, :n],
                                     start=(c == 0), stop=(c == last_c))
            rec = normp.tile([P, T], F32, tag="rec")
            pick.dve(1.0).reciprocal(rec, den)
            for dc in range(ndc):
                blk = blks[dc]
                if not is_b:
                    pick.dve(1.0).tensor_tensor(outacc[:, blk, :], yts[dc][:, :],
                                                rec, mult)
                else:
                    tmp = normp.tile([P, T], F32, tag="btmp")
                    pick.dve(1.0).tensor_tensor(tmp, yts[dc][:, :], rec, mult)
                    pick.tt(1.0).tensor_tensor(outacc[:, blk, :],
                                               outacc[:, blk, :], tmp, add)
                    nc.sync.dma_start(out=outT[P * blk:P * (blk + 1), :],
                                      in_=outacc[:, blk, :])

        # ================= config A =================
        for h in range(cfg.NA):
            q1 = rawp.tile([P, T], F32, tag="rS")
            nc.sync.dma_start(out=q1, in_=D["qT1"][P * h:P * (h + 1), :])
            q2 = rawp.tile([P, 2, T], F32, tag="rD")
            nc.sync.dma_start(out=q2, in_=D["qT2"][256 * h:256 * (h + 1), :]
                              .rearrange("(c p) t -> p c t", p=P))
            qmix = mixp.tile([P, 2, T], F32R, tag="qmix")
            mix_qk_A(qmix, q1, q2, tabs["ca1"], tabs["sa1"], tabs["ca2"], tabs["sa2"])

            k1 = rawp.tile([P, T], F32, tag="rS")
            nc.sync.dma_start(out=k1, in_=D["kTa1"][P * h:P * (h + 1), :])
            k2 = rawp.tile([P, 2, T], F32, tag="rD")
            nc.sync.dma_start(out=k2, in_=D["kTa2"][256 * h:256 * (h + 1), :]
                              .rearrange("(c p) t -> p c t", p=P))
            kmix = mixp.tile([P, 2, T], F32R, tag="kmix")
            mix_qk_A(kmix, k1, k2, tabs["ca1"], tabs["sa1"], tabs["ca2"], tabs["sa2"])

            v1 = rawp.tile([P, TK, P], F32, tag="rv1")
            nc.sync.dma_start(out=v1, in_=D["va1"][:, P * h:P * (h + 1)]
                              .rearrange("(c p) d -> p c d", p=P))
            v2 = rawp.tile([P, TK, 2 * P], F32, tag="rv2")
            nc.sync.dma_start(out=v2, in_=D["va2"][:, 2 * P * h:2 * P * (h + 1)]
                              .rearrange("(c p) d -> p c d", p=P))
            vmix = mixp.tile([P, TK, 2 * P], F32R, tag="vmix")
            pick.dve(2.0).tensor_scalar_mul(vmix, v2, wv[:, 1:2])
            pick.dve(1.0).scalar_tensor_tensor(
                out=vmix[:, :, 0:P], in0=v1, scalar=wv[:, 0:1],
                in1=vmix[:, :, 0:P], op0=mult, op1=add)

            attn_head([qmix[:, 0, :], qmix[:, 1, :]],
                      [kmix[:, 0, :], kmix[:, 1, :]],
                      vmix, (2 * h, 2 * h + 1), is_b=False)

        # ================= config B =================
        for j in range(cfg.NKVB):  # kv head j serves B-heads (2j, 2j+1)
            k2 = rawp.tile([P, T], F32, tag="rS")
            nc.sync.dma_start(out=k2, in_=D["kTb2"][P * j:P * (j + 1), :])
            # packed pair of d=64 kv slices: kv (2*(j//2)), (2*(j//2)+1)
            k1p = rawp.tile([P, T], F32, tag="rS")
            jp = j // 2
            nc.sync.dma_start(out=k1p, in_=D["kTb1"][P * jp:P * (jp + 1), :])

            kmix = mixp.tile([P, T], F32R, tag="kmix")
            t1 = scr.tile([P, T], F32, tag="t1")
            pick.tt().tensor_tensor(kmix, k2, tabs["cb2"][:, 0, :], mult)
            halfmul(t1, k2, tabs["sb2"][:, 0, :], 64)
            pick.tt().tensor_tensor(kmix, kmix, t1, add)
            # d=64 part only on rows 0:64 (uses half of the packed pair tile)
            half = 0 if j % 2 == 0 else 64
            sl = slice(half, half + 64)
            ts = scr.tile([P, T], F32, tag="t2", name="ts")
            pick.tt().tensor_tensor(ts[sl, :], k1p[sl, :],
                                    tabs["cb1"][sl, 0, :], mult)
            tb = scr.tile([P, T], F32, tag="t3", name="tb")
            halfmul(tb, k1p[sl, :], tabs["sb1"][sl, 0, :], 32, base=half, rows=64)
            pick.tt().tensor_tensor(ts[sl, :], ts[sl, :], tb[sl, :], add)
            if half == 0:
                pick.tt().tensor_tensor(kmix[0:64, :], kmix[0:64, :], ts[sl, :], add)
            else:
                xb_add(kmix[0:64, :], ts[sl, :], 1.0)

            v2 = rawp.tile([P, TK, P], F32, tag="rv1")
            nc.sync.dma_start(out=v2, in_=D["vb2"][:, P * j:P * (j + 1)]
                              .rearrange("(c p) d -> p c d", p=P))
            v1 = rawp.tile([P, TK, 64], F32, tag="rv2")
            nc.sync.dma_start(out=v1, in_=D["vb1"][:, 64 * j:64 * (j + 1)]
                              .rearrange("(c p) d -> p c d", p=P))
            vmix = mixp.tile([P, TK, P], F32R, tag="vmix")
            pick.dve(1.0).tensor_scalar_mul(vmix, v2, wv[:, 3:4])
            pick.dve(0.5).scalar_tensor_tensor(
                out=vmix[:, :, 0:64], in0=v1, scalar=wv[:, 2:3],
                in1=vmix[:, :, 0:64], op0=mult, op1=add)

            # q pair for heads (2j, 2j+1)
            q2p = rawp.tile([P, 2, T], F32, tag="rD")
            nc.sync.dma_start(out=q2p, in_=D["qT2"][256 * j:256 * (j + 1), :]
                              .rearrange("(c p) t -> p c t", p=P))
            q1p = rawp.tile([P, T], F32, tag="rS")
            nc.sync.dma_start(out=q1p, in_=D["qT1"][P * j:P * (j + 1), :])
            qmixp = mixp.tile([P, 2, T], F32R, tag="qmix")
            mix_qk_B_pair(qmixp, q1p, q2p, tabs["cb1"], tabs["sb1"],
                          tabs["cb2"], tabs["sb2"])

            for hh in range(2):
                b = 2 * j + hh
                attn_head([qmixp[:, hh, :]], [kmix], vmix, (b,), is_b=True)

    nc.compile()
    return nc


# ---------------------------------------------------------------------------
# Host side
# ---------------------------------------------------------------------------

def _rope_tab(pos, d, f):
    """Transposed rope tables [d, T]: (f*cos, +-f*sin with rot sign folded)."""
    inv = 1.0 / (10000.0 ** (np.arange(0, d, 2, dtype=np.float32) / d))
    ang = inv[:, None] * pos[None, :].astype(np.float32)      # [d/2, T]
    ang = np.concatenate([ang, ang], 0)                        # [d, T]
    c = (f * np.cos(ang)).astype(np.float32)
    s = (f * np.sin(ang)).astype(np.float32)
    s[: d // 2] *= -1.0
    return c, s


def make_core_inputs(q, k, v, pos, weights, s, cfg: KCfg = FULL):
    """q,k,v: [T, 2048] for one batch; returns the per-core input dict."""
    T = cfg.T
    c = np.ascontiguousarray
    arrs = {
        "qT1": c(q[:, 512 * s:512 * s + 512].T),
        "qT2": c(q[:, 1024 * s:1024 * s + 1024].T),
        "kTa1": c(k[:, 512 * s:512 * s + 512].T),
        "kTa2": c(k[:, 1024 * s:1024 * s + 1024].T),
        "kTb1": c(k[:, 256 * s:256 * s + 256].T),
        "kTb2": c(k[:, 512 * s:512 * s + 512].T),
        "va1": c(v[:, 512 * s:512 * s + 512]),
        "va2": c(v[:, 1024 * s:1024 * s + 1024]),
        "vb1": c(v[:, 256 * s:256 * s + 256]),
        "vb2": c(v[:, 512 * s:512 * s + 512]),
    }
    fA = math.sqrt(1.0 / 16.0)
    fB = math.sqrt(1.0 / math.sqrt(128.0))
    ca1, sa1 = _rope_tab(pos, 128, fA * float(weights[0]))
    ca2, sa2 = _rope_tab(pos, 256, fA * float(weights[1]))
    cb1h, sb1h = _rope_tab(pos, 64, fB * float(weights[2]))
    cb2, sb2 = _rope_tab(pos, 128, fB * float(weights[3]))

    def sigma(tab, half):
        # swap halves of size `half` within each 2*half row group
        out = tab.reshape(-1, 2, half, tab.shape[-1])
        return np.ascontiguousarray(
            out[:, ::-1].reshape(tab.shape))

    sb1 = np.vstack([sb1h, sb1h])
    arrs.update({
        # sin tables used through within-tile rotations are stored
        # sigma-permuted (device computes u = x * s_sigma, then rotates u
        # via cross-base copies); sa2 (d=256) rotates across tiles and
        # stays in math order.
        "ca1": ca1, "sa1": sigma(sa1, 64), "ca2": ca2, "sa2": sa2,
        "cb1": np.vstack([cb1h, cb1h]), "sb1": sigma(sb1, 32),
        "cb2": cb2, "sb2": sigma(sb2, 64),
        "wvec": np.tile(np.asarray(weights, np.float32)[None, :], (P, 1)),
        # math-order copies for numpy models (not used by the device)
        "_m_sa1": sa1, "_m_sb1": sb1, "_m_sb2": sb2,
    })
    return arrs


_PROGRAM_CACHE = {}
TRACE = False
LAST_RESULT = None


def kernel(q_m, k_m, v_m, weights, attention_mask, position_ids):
    global LAST_RESULT
    from concourse.bass_utils import run_bass_kernel_spmd

    cfg = FULL
    q_m = np.asarray(q_m, np.float32)
    k_m = np.asarray(k_m, np.float32)
    v_m = np.asarray(v_m, np.float32)
    weights = np.asarray(weights, np.float32)
    attention_mask = np.asarray(attention_mask, np.float32)
    position_ids = np.asarray(position_ids)
    B, T, H = q_m.shape

    # the device program hardcodes the causal structure; verify it holds
    causal = np.where(np.tril(np.ones((T, T), bool)), 0.0, NEG).astype(np.float32)
    for b in range(B):
        assert np.array_equal(attention_mask[b, 0], causal), "non-causal mask"

    if "nc" not in _PROGRAM_CACHE:
        _PROGRAM_CACHE["nc"] = build_program(cfg)
    nc = _PROGRAM_CACHE["nc"]

    in_maps = []
    for b in range(B):
        for s in range(2):
            in_maps.append(make_core_inputs(
                q_m[b], k_m[b], v_m[b], position_ids[b], weights, s, cfg))
    res = run_bass_kernel_spmd(nc, in_maps, list(range(8)), trace=TRACE)
    LAST_RESULT = res
    out = np.zeros((B, T, H), np.float32)
    for b in range(B):
        for s in range(2):
            out[b, :, 1024 * s:1024 * s + 1024] = res.results[2 * b + s]["outT"].T
    return out



# revision 10
# speedup vs baseline: 1.5893x; 1.5893x over previous
"""Trainium2 Bass kernel for nn_MixedAttnHeadEmbed (mixed-head-config attention).

Math (per batch b):
  Two attention configs share q_m/k_m/v_m [B,T,2048]:
    A: h=8  heads, d_max=256, mixing e in {1024,2048} -> d in {128,256}, weights w0,w1
    B: h=16 heads, d_max=128, mixing e in {1024,2048} -> d in {64,128},  weights w2,w3
  Each config: per-head q/k slices are RoPE'd, weight-summed (padded to d_max),
  GQA (8 kv heads), causal softmax attention; outputs of both configs sum.

Sharding: 8 cores = 4 batches x 2 shards. Shard s owns A-heads [4s,4s+4) and
B-heads [8s,8s+8) -> both write output columns [1024s, 1024s+1024) which are
summed on device; per-core output is the transposed block outT [1024, T] (fp16).

Device design notes (cost-model driven):
  * Everything on the elementwise path is fp16: DVE tensor_tensor gets the
    2x_1p fast mode, tensor_copy/tensor_scalar get 4x, DMA bytes halve, and
    fp16 matmuls stream at 1 cycle/row at ANY output width (fp32r pays 4x
    under 256).  Accumulation stays fp32 in PSUM; softmax input is fp32.
  * rotate_half operands arrive pre-permuted from HBM (sigma64/sigma32 row
    permutations are folded into extra DMA loads) so RoPE needs no on-chip
    shuffles or cross-partition copies: each mix is pure mult/add passes.
  * Scores are computed transposed (sT[k,q]) so softmax'd weights feed the
    y^T matmul directly; softmax is max-free (scores provably < 2), the
    denominator comes from an all-ones stationary matmul, and the causal
    diagonal is enforced by zeroing exp() outputs with affine_select (Pool,
    SBUF) instead of adding -1e9 to PSUM scores (DVE).
  * PSUM budget (8 banks): sT [P,1024]f32 double-buffered (4) + y (2) +
    den (2).  Config-A heads keep exp() outputs pt_c in SBUF and run the
    second output-d-chunk as a pure-matmul second pass over them.
  * ACT drains y PSUM tiles to SBUF fp16 copies so the single y region is
    released to the next accumulation chain at copy speed; DVE only does
    reciprocal + cheap fp16 normalize multiplies.
  * A ns-calibrated static balancer spreads mix passes across DVE/Pool
    (and ACT for copies) around the reserved exp/mask/normalize work.
"""

import math
from contextlib import ExitStack
from dataclasses import dataclass

import numpy as np

import concourse.bass as bass
import concourse.mybir as mybir
import concourse.tile as tile
from concourse import bacc

F32 = mybir.dt.float32
F16 = mybir.dt.float16
NEG = -1e9
P = 128


@dataclass(frozen=True)
class KCfg:
    T: int = 1024       # sequence length
    NA: int = 4         # config-A heads per core (d_max=256)
    NB: int = 8         # config-B heads per core (d_max=128); must be 2*NA

    @property
    def TK(self):
        return self.T // P

    @property
    def NKVB(self):
        return self.NB // 2

    @property
    def ROWS(self):
        return self.NA * 256  # == NB * 128 output rows per core


FULL = KCfg()


def _in_specs(cfg: KCfg):
    T = cfg.T
    na, nb = cfg.NA, cfg.NKVB
    return {
        # q/k transposed [cols, T]; *_s64/_s32 are rotate-half row permutations
        "qT1": (na * 128, T), "qT1s64": (na * 128, T), "qT1s32": (na * 128, T),
        "qT2": (na * 256, T), "qT2s64": (na * 256, T),
        "kTa1": (na * 128, T), "kTa1s64": (na * 128, T),
        "kTa2": (na * 256, T),
        "kTb1": (nb * 64, T), "kTb1s32": (nb * 64, T),
        # v pre-permuted per head: rows = head*P + p, cols = (chunk, d) flat
        "va1p": (na * P, (T // P) * 128),
        "va2p": (na * P, (T // P) * 256),
        "vb1p": (nb * P, (T // P) * 64),
        # rope tables (weights & score scale folded, rot sign folded in sin)
        "tA1c": (128, T), "tA1s": (128, T),
        "tA2c": (256, T), "tA2s": (256, T),   # tA2s is half-SWAPPED (see host)
        "tB1c": (128, T), "tB1s": (128, T),
        "tB2c": (128, T), "tB2s": (128, T),
        "wvec": (P, 4),
    }


class _Pick:
    """Static ns-accurate load balancer across DVE / Pool(GPSIMD) / ACT.

    v1 cost model: engine time = free_size * cycle_t (DVE 1/0.96GHz, Pool &
    ACT 1/1.2GHz) with DVE fast modes: fp16 TT 2x, fp16 sbuf copy/TSP 4x.
    Init adders: DVE +60ns sbuf / +125ns psum, ACT +185ns, Pool ~+40ns."""

    def __init__(self, nc):
        self.nc = nc
        self.load = {"dve": 0.0, "pool": 0.0, "act": 0.0}

    # --- reservations for work that must sit on one engine ---
    def act_reserve(self, ns):
        self.load["act"] += ns
        return self.nc.scalar

    def pool_reserve(self, ns):
        self.load["pool"] += ns
        return self.nc.gpsimd

    def dve_reserve(self, ns):
        self.load["dve"] += ns
        return self.nc.vector

    # --- balanced ops ---
    def tt(self, out, a, b, op, free, fast=True, psum=False):
        d = free * 1.0417 * (0.5 if (fast and not psum) else 1.0) + (125 if psum else 60)
        if psum:
            self.load["dve"] += d
            self.nc.vector.tensor_tensor(out, a, b, op)
            return
        p = free * 0.8333 + 40
        if self.load["dve"] + d <= self.load["pool"] + p:
            self.load["dve"] += d
            self.nc.vector.tensor_tensor(out, a, b, op)
        else:
            self.load["pool"] += p
            self.nc.gpsimd.tensor_tensor(out, a, b, op)

    def copy(self, dst, src, free):
        costs = {"dve": free * 0.2604 + 60, "pool": free * 0.8333 + 40,
                 "act": free * 0.8333 + 217}
        eng = min(costs, key=lambda k: self.load[k] + costs[k])
        self.load[eng] += costs[eng]
        if eng == "act":
            self.nc.scalar.copy(dst, src)
        elif eng == "pool":
            self.nc.gpsimd.tensor_copy(dst, src)
        else:
            self.nc.vector.tensor_copy(dst, src)

    def tsp_mul(self, out, in0, scalar, free):
        self.load["dve"] += free * 0.2604 + 60
        self.nc.vector.tensor_scalar_mul(out, in0, scalar)


def build_program(cfg: KCfg = FULL):
    nc = bacc.Bacc("TRN2", target_bir_lowering=False)
    T, TK = cfg.T, cfg.TK
    mult, add = mybir.AluOpType.mult, mybir.AluOpType.add

    D = {}
    for name, shape in _in_specs(cfg).items():
        dt = F32 if name == "wvec" else F16
        D[name] = nc.declare_dram_parameter(name, list(shape), dt, isOutput=False)
    outT = nc.declare_dram_parameter("outT", [cfg.ROWS, T], F16, isOutput=True)
    RB = cfg.ROWS // P

    with ExitStack() as ctx:
        tc = ctx.enter_context(tile.TileContext(nc))
        const = ctx.enter_context(tc.tile_pool(name="const", bufs=1))
        rawp = ctx.enter_context(tc.tile_pool(name="raw", bufs=2))
        mixp = ctx.enter_context(tc.tile_pool(name="mix", bufs=2))
        scr = ctx.enter_context(tc.tile_pool(name="scr", bufs=2))
        ptp = ctx.enter_context(tc.tile_pool(name="pt", bufs=2))
        ycp = ctx.enter_context(tc.tile_pool(name="yc", bufs=2))
        recp = ctx.enter_context(tc.tile_pool(name="rec", bufs=2))
        accp = ctx.enter_context(tc.tile_pool(name="acc", bufs=1))
        spsum = ctx.enter_context(tc.tile_pool(name="spsum", bufs=2, space="PSUM"))
        ypsum = ctx.enter_context(tc.tile_pool(name="ypsum", bufs=1, space="PSUM"))
        dpsum = ctx.enter_context(tc.tile_pool(name="dpsum", bufs=1, space="PSUM"))

        pick = _Pick(nc)

        # ---- constants ----
        ones_f = const.tile([P, P], F32, name="ones_f")
        nc.vector.memset(ones_f, 1.0)
        ones = const.tile([P, P], F16)
        nc.vector.tensor_copy(ones, ones_f)
        tabs = {}
        for nm in ("tA1c", "tA1s", "tA2c", "tA2s", "tB1c", "tB1s", "tB2c", "tB2s"):
            rows = _in_specs(cfg)[nm][0]
            tl = const.tile([P, rows // P, T], F16, name=nm, tag=nm)
            tabs[nm] = tl
            nc.sync.dma_start(out=tl, in_=D[nm].rearrange("(c p) t -> p c t", p=P))
        wv = const.tile([P, 4], F32)
        nc.sync.dma_start(out=wv, in_=D["wvec"][:, :])

        outacc = accp.tile([P, RB, T], F16)

        EXP = mybir.ActivationFunctionType.Exp

        def mix_A(out, x1, x1s, x2, c1, s1, c2, s2sw):
            """out [P,2,T] f16 = RoPE-mix of a config-A q or k head.
            x2 [P,2,T] (d=256), x1/x1s [P,T] (d=128, x1s sigma64-permuted).
            s2sw is the half-swapped signed d=256 sin table."""
            u2 = scr.tile([P, 2, T], F16, tag="u2")
            u1 = scr.tile([P, T], F16, tag="u1")
            u1b = scr.tile([P, T], F16, tag="u1b")
            pick.tt(out, x2, c2, mult, 2 * T)          # aligned cos products
            pick.tt(u2, x2, s2sw, mult, 2 * T)         # swapped sin products
            pick.tt(out[:, 0, :], out[:, 0, :], u2[:, 1, :], add, T)
            pick.tt(out[:, 1, :], out[:, 1, :], u2[:, 0, :], add, T)
            pick.tt(u1, x1, c1[:, 0, :], mult, T)
            pick.tt(u1b, x1s, s1[:, 0, :], mult, T)
            pick.tt(out[:, 0, :], out[:, 0, :], u1, add, T)
            pick.tt(out[:, 0, :], out[:, 0, :], u1b, add, T)

        def attn(q_aps, k_aps, v_lhs, blks, is_b):
            """q_aps/k_aps: per-d-chunk [P,T] f16 APs (d on partitions).
            v_lhs(c, vc): stationary [P, d] AP for k-chunk c, out-chunk vc.
            blks: output 128-row blocks (1 for B, 2 for A)."""
            ndc = len(q_aps)
            BW = 512  # PSUM bank width in f32: matmul outs must stay in-bank

            def pieces(c):
                q0 = P * c
                return [(r, max(BW * r, q0), BW * (r + 1))
                        for r in range(T // BW) if BW * (r + 1) > max(BW * r, q0)]

            def last_c(r):
                return min(TK, (BW // P) * (r + 1)) - 1

            den = dpsum.tile([P, T], F32, tag="den")
            yt = ypsum.tile([P, T], F32, tag="yt")
            pts = []
            for c in range(TK):
                q0 = P * c
                n = T - q0
                sT = spsum.tile([P, T], F32, tag="sT")
                for (r, lo, hi) in pieces(c):
                    for dc in range(ndc):
                        nc.tensor.matmul(
                            sT[:, lo:hi], k_aps[dc][:, q0:q0 + P], q_aps[dc][:, lo:hi],
                            start=(dc == 0), stop=(dc == ndc - 1))
                pt = ptp.tile([P, n], F16, tag=f"pt{c}", name=f"pt{c}")
                pick.act_reserve(n * 0.8333 + 185).activation(pt, sT[:, q0:], EXP)
                # causal diagonal: zero exp() where q < k inside the diag block
                pick.pool_reserve(150).affine_select(
                    out=pt[:, 0:P], in_=pt[:, 0:P],
                    compare_op=mybir.AluOpType.is_ge, fill=0.0,
                    base=0, pattern=[[1, P]], channel_multiplier=-1)
                for (r, lo, hi) in pieces(c):
                    nc.tensor.matmul(yt[:, lo:hi], v_lhs(c, 0), pt[:, lo - q0:hi - q0],
                                     start=(c == 0), stop=(c == last_c(r)))
                    nc.tensor.matmul(den[:, lo:hi], ones, pt[:, lo - q0:hi - q0],
                                     start=(c == 0), stop=(c == last_c(r)))
                pts.append(pt)
            yc0 = ycp.tile([P, T], F16, tag="yc")
            pick.act_reserve(T * 0.8333 + 217).copy(yc0, yt)
            rec = recp.tile([P, T], F16, tag="rec")
            with nc.allow_low_precision(reason="1/den fits fp16; den in [1, 8e3]"):
                pick.dve_reserve(T * 1.0417 + 125).reciprocal(rec, den)
            if not is_b:
                yt2 = ypsum.tile([P, T], F32, tag="yt")
                for c in range(TK):
                    q0 = P * c
                    for (r, lo, hi) in pieces(c):
                        nc.tensor.matmul(yt2[:, lo:hi], v_lhs(c, 1),
                                         pts[c][:, lo - q0:hi - q0],
                                         start=(c == 0), stop=(c == last_c(r)))
                yc1 = ycp.tile([P, T], F16, tag="yc")
                pick.act_reserve(T * 0.8333 + 217).copy(yc1, yt2)
                pick.tt(outacc[:, blks[0], :], yc0, rec, mult, T)
                pick.tt(outacc[:, blks[1], :], yc1, rec, mult, T)
            else:
                tmp = scr.tile([P, T], F16, tag="btmp")
                pick.tt(tmp, yc0, rec, mult, T)
                pick.tt(outacc[:, blks[0], :], outacc[:, blks[0], :], tmp, add, T)
                nc.sync.dma_start(out=outT[P * blks[0]:P * (blks[0] + 1), :],
                                  in_=outacc[:, blks[0], :])

        # ================= config A =================
        for i in range(cfg.NA):
            r1, r1s = slice(P * i, P * (i + 1)), slice(P * i, P * (i + 1))
            r2 = slice(256 * i, 256 * (i + 1))
            q1 = rawp.tile([P, T], F16, tag="q1")
            nc.sync.dma_start(out=q1, in_=D["qT1"][r1, :])
            q1s = rawp.tile([P, T], F16, tag="q1s")
            nc.sync.dma_start(out=q1s, in_=D["qT1s64"][r1s, :])
            q2 = rawp.tile([P, 2, T], F16, tag="q2")
            nc.sync.dma_start(out=q2, in_=D["qT2"][r2, :].rearrange("(c p) t -> p c t", p=P))
            qmix = mixp.tile([P, 2, T], F16, tag="qmix")
            mix_A(qmix, q1, q1s, q2, tabs["tA1c"], tabs["tA1s"], tabs["tA2c"], tabs["tA2s"])

            k1 = rawp.tile([P, T], F16, tag="k1")
            nc.sync.dma_start(out=k1, in_=D["kTa1"][r1, :])
            k1s = rawp.tile([P, T], F16, tag="k1s")
            nc.sync.dma_start(out=k1s, in_=D["kTa1s64"][r1s, :])
            k2 = rawp.tile([P, 2, T], F16, tag="k2")
            nc.sync.dma_start(out=k2, in_=D["kTa2"][r2, :].rearrange("(c p) t -> p c t", p=P))
            kmix = mixp.tile([P, 2, T], F16, tag="kmix")
            mix_A(kmix, k1, k1s, k2, tabs["tA1c"], tabs["tA1s"], tabs["tA2c"], tabs["tA2s"])

            v1 = rawp.tile([P, TK, P], F16, tag="v1")
            nc.sync.dma_start(out=v1, in_=D["va1p"][r1, :].rearrange("p (c d) -> p c d", d=P))
            v2 = rawp.tile([P, TK, 2 * P], F16, tag="v2")
            nc.sync.dma_start(out=v2, in_=D["va2p"][r1, :].rearrange("p (c d) -> p c d", d=2 * P))
            vmix = mixp.tile([P, TK, 2 * P], F16, tag="vmix")
            pick.tsp_mul(vmix, v2, wv[:, 1:2], 2 * T)
            u = scr.tile([P, TK, P], F16, tag="vu")
            pick.tsp_mul(u, v1, wv[:, 0:1], T)
            pick.tt(vmix[:, :, 0:P], vmix[:, :, 0:P], u, add, T)

            attn([qmix[:, 0, :], qmix[:, 1, :]],
                 [kmix[:, 0, :], kmix[:, 1, :]],
                 lambda c, vc: vmix[:, c, P * vc:P * (vc + 1)],
                 (2 * i, 2 * i + 1), is_b=False)

        # ================= config B =================
        for j in range(cfg.NKVB):  # kv head j serves B-heads (2j, 2j+1)
            rj = slice(P * j, P * (j + 1))
            k2 = rawp.tile([P, T], F16, tag="q1")
            nc.sync.dma_start(out=k2, in_=D["kTa1"][rj, :])
            k2s = rawp.tile([P, T], F16, tag="q1s")
            nc.sync.dma_start(out=k2s, in_=D["kTa1s64"][rj, :])
            kmix = mixp.tile([P, T], F16, tag="bkmix")
            u = scr.tile([P, T], F16, tag="u1")
            pick.tt(kmix, k2, tabs["tB2c"][:, 0, :], mult, T)
            pick.tt(u, k2s, tabs["tB2s"][:, 0, :], mult, T)
            pick.tt(kmix, kmix, u, add, T)
            if j % 2 == 0:
                # packed d=64 kv pair (kv j, j+1): rows [128u0, 128u0+128)
                u0 = j // 2
                k1p = rawp.tile([P, T], F16, tag="bk1p")
                nc.sync.dma_start(out=k1p, in_=D["kTb1"][P * u0:P * (u0 + 1), :])
                k1ps = rawp.tile([P, T], F16, tag="bk1ps")
                nc.sync.dma_start(out=k1ps, in_=D["kTb1s32"][P * u0:P * (u0 + 1), :])
                tp = scr.tile([P, T], F16, tag="btp")
                tpb = scr.tile([P, T], F16, tag="btpb")
                pick.tt(tp, k1p, tabs["tB1c"][:, 0, :], mult, T)
                pick.tt(tpb, k1ps, tabs["tB1s"][:, 0, :], mult, T)
                pick.tt(tp, tp, tpb, add, T)
                last_tp = tp
                pick.tt(kmix[0:64, :], kmix[0:64, :], tp[0:64, :], add, T)
            else:
                tc2 = scr.tile([P, T], F16, tag="btc")
                pick.copy(tc2[0:64, :], last_tp[64:128, :], T)
                pick.tt(kmix[0:64, :], kmix[0:64, :], tc2[0:64, :], add, T)

            v2 = rawp.tile([P, TK, P], F16, tag="v1")
            nc.sync.dma_start(out=v2, in_=D["va1p"][rj, :].rearrange("p (c d) -> p c d", d=P))
            v1 = rawp.tile([P, TK, 64], F16, tag="bv1")
            nc.sync.dma_start(out=v1, in_=D["vb1p"][rj, :].rearrange("p (c d) -> p c d", d=64))
            vmix = mixp.tile([P, TK, P], F16, tag="bvmix")
            pick.tsp_mul(vmix, v2, wv[:, 3:4], T)
            uv = scr.tile([P, TK, 64], F16, tag="bvu")
            pick.tsp_mul(uv, v1, wv[:, 2:3], T // 2)
            pick.tt(vmix[:, :, 0:64], vmix[:, :, 0:64], uv, add, T // 2)

            # q pair for heads (2j, 2j+1)
            q2p = rawp.tile([P, 2, T], F16, tag="q2")
            nc.sync.dma_start(out=q2p, in_=D["qT2"][256 * j:256 * (j + 1), :]
                              .rearrange("(c p) t -> p c t", p=P))
            q2ps = rawp.tile([P, 2, T], F16, tag="k2")
            nc.sync.dma_start(out=q2ps, in_=D["qT2s64"][256 * j:256 * (j + 1), :]
                              .rearrange("(c p) t -> p c t", p=P))
            q1p = rawp.tile([P, T], F16, tag="k1")
            nc.sync.dma_start(out=q1p, in_=D["qT1"][rj, :])
            q1ps = rawp.tile([P, T], F16, tag="k1s")
            nc.sync.dma_start(out=q1ps, in_=D["qT1s32"][rj, :])

            qp = mixp.tile([P, 2, T], F16, tag="bqp")
            uq = scr.tile([P, 2, T], F16, tag="u2")
            for hh in range(2):
                pick.tt(qp[:, hh, :], q2p[:, hh, :], tabs["tB2c"][:, 0, :], mult, T)
                pick.tt(uq[:, hh, :], q2ps[:, hh, :], tabs["tB2s"][:, 0, :], mult, T)
            pick.tt(qp, qp, uq, add, 2 * T)
            t1 = scr.tile([P, T], F16, tag="u1")
            t1b = scr.tile([P, T], F16, tag="u1b")
            pick.tt(t1, q1p, tabs["tB1c"][:, 0, :], mult, T)
            pick.tt(t1b, q1ps, tabs["tB1s"][:, 0, :], mult, T)
            pick.tt(t1, t1, t1b, add, T)
            pick.tt(qp[0:64, 0, :], qp[0:64, 0, :], t1[0:64, :], add, T)
            tcq = scr.tile([P, T], F16, tag="btc")
            pick.copy(tcq[0:64, :], t1[64:128, :], T)
            pick.tt(qp[0:64, 1, :], qp[0:64, 1, :], tcq[0:64, :], add, T)

            for hh in range(2):
                attn([qp[:, hh, :]], [kmix],
                     lambda c, vc: vmix[:, c, :],
                     (2 * j + hh,), is_b=True)

    nc.compile()
    return nc


# ---------------------------------------------------------------------------
# Host side
# ---------------------------------------------------------------------------

def _rope_tab(pos, d, f):
    """Transposed rope tables [d, T]: (f*cos, f*sin with rot sign folded)."""
    inv = 1.0 / (10000.0 ** (np.arange(0, d, 2, dtype=np.float32) / d))
    ang = inv[:, None] * pos[None, :].astype(np.float32)      # [d/2, T]
    ang = np.concatenate([ang, ang], 0)                        # [d, T]
    c = (f * np.cos(ang)).astype(np.float32)
    s = (f * np.sin(ang)).astype(np.float32)
    s[: d // 2] *= -1.0
    return c, s


def _sig(a, half):
    """Row permutation: swap halves of size `half` in each 2*half group."""
    out = a.reshape(-1, 2, half, a.shape[-1])[:, ::-1]
    return out.reshape(a.shape)


def _vperm(vslc, dh):
    """[T, nh*dh] -> [nh, P, T//P, dh] contiguous per-partition rows."""
    T = vslc.shape[0]
    nh = vslc.shape[1] // dh
    # [c, p, head, d] -> [head, p, c, d]
    return vslc.reshape(T // P, P, nh, dh).transpose(2, 1, 0, 3)


def make_core_inputs(q, k, v, pos, weights, s, cfg: KCfg = FULL):
    """q,k,v: [T, 2048] fp32 for one batch; returns the per-core input dict."""
    f16 = lambda a: np.ascontiguousarray(a, dtype=np.float16)
    qT1 = q[:, 512 * s:512 * s + 512].T
    qT2 = q[:, 1024 * s:1024 * s + 1024].T
    kTa1 = k[:, 512 * s:512 * s + 512].T
    kTa2 = k[:, 1024 * s:1024 * s + 1024].T
    kTb1 = k[:, 256 * s:256 * s + 256].T
    arrs = {
        "qT1": f16(qT1), "qT1s64": f16(_sig(qT1, 64)), "qT1s32": f16(_sig(qT1, 32)),
        "qT2": f16(qT2), "qT2s64": f16(_sig(qT2, 64)),
        "kTa1": f16(kTa1), "kTa1s64": f16(_sig(kTa1, 64)),
        "kTa2": f16(kTa2),
        "kTb1": f16(kTb1), "kTb1s32": f16(_sig(kTb1, 32)),
        "va1p": f16(_vperm(v[:, 512 * s:512 * s + 512], 128).reshape(4 * P, -1)),
        "va2p": f16(_vperm(v[:, 1024 * s:1024 * s + 1024], 256).reshape(4 * P, -1)),
        "vb1p": f16(_vperm(v[:, 256 * s:256 * s + 256], 64).reshape(4 * P, -1)),
    }
    fA = math.sqrt(1.0 / 16.0)
    fB = math.sqrt(1.0 / math.sqrt(128.0))
    c1, s1 = _rope_tab(pos, 128, fA * float(weights[0]))
    c2, s2 = _rope_tab(pos, 256, fA * float(weights[1]))
    cb1h, sb1h = _rope_tab(pos, 64, fB * float(weights[2]))
    cb2, sb2 = _rope_tab(pos, 128, fB * float(weights[3]))
    arrs.update({
        "tA1c": f16(c1), "tA1s": f16(s1),
        # tA2s half-swapped: row block 0 holds the sin factors for x2[:,0,:]
        # (which contribute to out dim-chunk 1), block 1 those for x2[:,1,:].
        "tA2c": f16(c2), "tA2s": f16(np.vstack([s2[128:], s2[:128]])),
        "tB1c": f16(np.vstack([cb1h, cb1h])), "tB1s": f16(np.vstack([sb1h, sb1h])),
        "tB2c": f16(cb2), "tB2s": f16(sb2),
        "wvec": np.tile(np.asarray(weights, np.float32)[None, :], (P, 1)),
    })
    return arrs


_PROGRAM_CACHE = {}
TRACE = False
LAST_RESULT = None


def kernel(q_m, k_m, v_m, weights, attention_mask, position_ids):
    global LAST_RESULT
    from concourse.bass_utils import run_bass_kernel_spmd

    cfg = FULL
    q_m = np.asarray(q_m, np.float32)
    k_m = np.asarray(k_m, np.float32)
    v_m = np.asarray(v_m, np.float32)
    weights = np.asarray(weights, np.float32)
    attention_mask = np.asarray(attention_mask, np.float32)
    position_ids = np.asarray(position_ids)
    B, T, H = q_m.shape

    # the device program hardcodes the causal structure; verify it holds
    causal = np.where(np.tril(np.ones((T, T), bool)), 0.0, NEG).astype(np.float32)
    for b in range(B):
        assert np.array_equal(attention_mask[b, 0], causal), "non-causal mask"

    if "nc" not in _PROGRAM_CACHE:
        _PROGRAM_CACHE["nc"] = build_program(cfg)
    nc = _PROGRAM_CACHE["nc"]

    in_maps = []
    for b in range(B):
        for s in range(2):
            in_maps.append(make_core_inputs(
                q_m[b], k_m[b], v_m[b], position_ids[b], weights, s, cfg))
    res = run_bass_kernel_spmd(nc, in_maps, list(range(8)), trace=TRACE)
    LAST_RESULT = res
    out = np.zeros((B, T, H), np.float32)
    for b in range(B):
        for s in range(2):
            out[b, :, 1024 * s:1024 * s + 1024] = \
                res.results[2 * b + s]["outT"].astype(np.float32).T
    return out


# revision 16
# speedup vs baseline: 1.6339x; 1.0281x over previous
"""Trainium2 Bass kernel for nn_MixedAttnHeadEmbed (mixed-head-config attention).

Math (per batch b):
  Two attention configs share q_m/k_m/v_m [B,T,2048]:
    A: h=8  heads, d_max=256, mixing e in {1024,2048} -> d in {128,256}, weights w0,w1
    B: h=16 heads, d_max=128, mixing e in {1024,2048} -> d in {64,128},  weights w2,w3
  Each config: per-head q/k slices are RoPE'd, weight-summed (padded to d_max),
  GQA (8 kv heads), causal softmax attention; outputs of both configs sum.

Sharding: 8 cores = 4 batches x 2 shards. Shard s owns A-heads [4s,4s+4) and
B-heads [8s,8s+8) -> both write output columns [1024s, 1024s+1024) which are
summed on device; per-core output is the transposed block outT [1024, T] (fp16).

Device design notes (cost-model driven):
  * Everything on the elementwise path is fp16: DVE tensor_tensor gets the
    2x_1p fast mode, tensor_copy/tensor_scalar get 4x, DMA bytes halve, and
    fp16 matmuls stream at 1 cycle/row at ANY output width (fp32r pays 4x
    under 256).  Accumulation stays fp32 in PSUM; softmax input is fp32.
  * rotate_half operands arrive pre-permuted from HBM (sigma64/sigma32 row
    permutations are folded into extra DMA loads) so RoPE needs no on-chip
    shuffles or cross-partition copies: each mix is pure mult/add passes.
  * Scores are computed transposed (sT[k,q]) so softmax'd weights feed the
    y^T matmul directly; softmax is max-free (scores provably < 2), the
    denominator comes from an all-ones stationary matmul, and the causal
    diagonal is enforced by zeroing exp() outputs with affine_select (Pool,
    SBUF) instead of adding -1e9 to PSUM scores (DVE).
  * PSUM budget (8 banks): sT [P,1024]f32 double-buffered (4) + y (2) +
    den (2).  Config-A heads keep exp() outputs pt_c in SBUF and run the
    second output-d-chunk as a pure-matmul second pass over them.
  * ACT drains y PSUM tiles to SBUF fp16 copies so the single y region is
    released to the next accumulation chain at copy speed; DVE only does
    reciprocal + cheap fp16 normalize multiplies.
  * A ns-calibrated static balancer spreads mix passes across DVE/Pool
    (and ACT for copies) around the reserved exp/mask/normalize work.
"""

import math
from contextlib import ExitStack
from dataclasses import dataclass

import numpy as np

import concourse.bass as bass
import concourse.mybir as mybir
import concourse.tile as tile
from concourse import bacc

F32 = mybir.dt.float32
F16 = mybir.dt.float16
NEG = -1e9
P = 128


@dataclass(frozen=True)
class KCfg:
    T: int = 1024       # sequence length
    NA: int = 4         # config-A heads per core (d_max=256)
    NB: int = 8         # config-B heads per core (d_max=128); must be 2*NA

    @property
    def TK(self):
        return self.T // P

    @property
    def NKVB(self):
        return self.NB // 2

    @property
    def ROWS(self):
        return self.NA * 256  # == NB * 128 output rows per core


FULL = KCfg()


def _in_specs(cfg: KCfg):
    T = cfg.T
    na, nb = cfg.NA, cfg.NKVB
    return {
        # q/k transposed [cols, T]; *_s64/_s32 are rotate-half row permutations
        "qT1": (na * 128, T), "qT1s64": (na * 128, T), "qT1s32": (na * 128, T),
        "qT2": (na * 256, T), "qT2s64": (na * 256, T),
        "kTa1": (na * 128, T), "kTa1s64": (na * 128, T),
        "kTa2": (na * 256, T),
        "kTb1": (nb * 64, T), "kTb1s32": (nb * 64, T),
        # v pre-permuted per head: rows = head*P + p, cols = (chunk, d) flat
        "va1p": (na * P, (T // P) * 128),
        "va2p": (na * P, (T // P) * 256),
        "vb1p": (nb * P, (T // P) * 64),
        # rope tables (weights & score scale folded, rot sign folded in sin)
        "tA1c": (128, T), "tA1s": (128, T),
        "tA2c": (256, T), "tA2s": (256, T),   # tA2s is half-SWAPPED (see host)
        "tB1c": (128, T), "tB1s": (128, T),
        "tB2c": (128, T), "tB2s": (128, T),
        "wvec": (P, 4),
    }


class _Pick:
    """Static ns-accurate load balancer across DVE / Pool(GPSIMD) / ACT.

    v1 cost model: engine time = free_size * cycle_t (DVE 1/0.96GHz, Pool &
    ACT 1/1.2GHz) with DVE fast modes: fp16 TT 2x, fp16 sbuf copy/TSP 4x.
    Init adders: DVE +60ns sbuf / +125ns psum, ACT +185ns, Pool ~+40ns."""

    def __init__(self, nc):
        self.nc = nc
        self.load = {"dve": 0.0, "pool": 0.0, "act": 0.0}

    # --- reservations for work that must sit on one engine ---
    def act_reserve(self, ns):
        self.load["act"] += ns
        return self.nc.scalar

    def pool_reserve(self, ns):
        self.load["pool"] += ns
        return self.nc.gpsimd

    def dve_reserve(self, ns):
        self.load["dve"] += ns
        return self.nc.vector

    # --- balanced ops ---
    def tt(self, out, a, b, op, free, fast=True, psum=False):
        d = free * 1.0417 * (0.5 if (fast and not psum) else 1.0) + (125 if psum else 60)
        if psum:
            self.load["dve"] += d
            self.nc.vector.tensor_tensor(out, a, b, op)
            return
        p = free * 0.8333 + 40
        if self.load["dve"] + d <= self.load["pool"] + p:
            self.load["dve"] += d
            self.nc.vector.tensor_tensor(out, a, b, op)
        else:
            self.load["pool"] += p
            self.nc.gpsimd.tensor_tensor(out, a, b, op)

    def copy(self, dst, src, free):
        costs = {"dve": free * 0.2604 + 60, "pool": free * 0.8333 + 40,
                 "act": free * 0.8333 + 217}
        eng = min(costs, key=lambda k: self.load[k] + costs[k])
        self.load[eng] += costs[eng]
        if eng == "act":
            self.nc.scalar.copy(dst, src)
        elif eng == "pool":
            self.nc.gpsimd.tensor_copy(dst, src)
        else:
            self.nc.vector.tensor_copy(dst, src)

    def tsp_mul(self, out, in0, scalar, free):
        self.load["dve"] += free * 0.2604 + 60
        self.nc.vector.tensor_scalar_mul(out, in0, scalar)

    def psum_drain(self, dst, src, free):
        """Copy a PSUM f32 tile to SBUF fp16: DVE or ACT (no fast modes)."""
        dcost = free * 1.0417 + 125
        acost = free * 0.8333 + 217
        if self.load["dve"] + dcost <= self.load["act"] + acost:
            self.load["dve"] += dcost
            self.nc.vector.tensor_copy(dst, src)
        else:
            self.load["act"] += acost
            self.nc.scalar.copy(dst, src)

    def tt_psum(self, out, a, b, op, free):
        """2-input op with a PSUM operand: DVE only."""
        self.load["dve"] += free * 1.0417 + 125
        self.nc.vector.tensor_tensor(out, a, b, op)


def build_program(cfg: KCfg = FULL):
    nc = bacc.Bacc("TRN2", target_bir_lowering=False)
    T, TK = cfg.T, cfg.TK
    mult, add = mybir.AluOpType.mult, mybir.AluOpType.add

    D = {}
    for name, shape in _in_specs(cfg).items():
        dt = F32 if name == "wvec" else F16
        D[name] = nc.declare_dram_parameter(name, list(shape), dt, isOutput=False)
    outT = nc.declare_dram_parameter("outT", [cfg.ROWS, T], F16, isOutput=True)
    RB = cfg.ROWS // P

    with ExitStack() as ctx:
        tc = ctx.enter_context(tile.TileContext(nc))
        const = ctx.enter_context(tc.tile_pool(name="const", bufs=1))
        rawp = ctx.enter_context(tc.tile_pool(name="raw", bufs=2))
        mixp = ctx.enter_context(tc.tile_pool(name="mix", bufs=2))
        scr = ctx.enter_context(tc.tile_pool(name="scr", bufs=2))
        ptp = ctx.enter_context(tc.tile_pool(name="pt", bufs=2))
        ycp = ctx.enter_context(tc.tile_pool(name="yc", bufs=2))
        recp = ctx.enter_context(tc.tile_pool(name="rec", bufs=2))
        accp = ctx.enter_context(tc.tile_pool(name="acc", bufs=1))
        spsum = ctx.enter_context(tc.tile_pool(name="spsum", bufs=2, space="PSUM"))
        ypsum = ctx.enter_context(tc.tile_pool(name="ypsum", bufs=1, space="PSUM"))
        dpsum = ctx.enter_context(tc.tile_pool(name="dpsum", bufs=1, space="PSUM"))

        pick = _Pick(nc)

        # ---- constants ----
        ones_f = const.tile([P, P], F32, name="ones_f")
        nc.vector.memset(ones_f, 1.0)
        ones = const.tile([P, P], F16)
        nc.vector.tensor_copy(ones, ones_f)
        tabs = {}

        def load_tab(nm):
            rows = _in_specs(cfg)[nm][0]
            tl = const.tile([P, rows // P, T], F16, name=nm, tag=nm)
            tabs[nm] = tl
            nc.sync.dma_start(out=tl, in_=D[nm].rearrange("(c p) t -> p c t", p=P))

        # A-phase tables only: B tables load later, off the warmup critical path
        for nm in ("tA1c", "tA1s", "tA2c", "tA2s"):
            load_tab(nm)
        wv = const.tile([P, 4], F32)
        nc.sync.dma_start(out=wv, in_=D["wvec"][:, :])

        outacc = accp.tile([P, RB, T], F16)

        EXP = mybir.ActivationFunctionType.Exp

        def mix_A(out, x1, x1s, x2, c1, s1, c2, s2sw):
            """out [P,2,T] f16 = RoPE-mix of a config-A q or k head.
            x2 [P,2,T] (d=256), x1/x1s [P,T] (d=128, x1s sigma64-permuted).
            s2sw is the half-swapped signed d=256 sin table."""
            u2 = scr.tile([P, 2, T], F16, tag="u2")
            u1 = scr.tile([P, T], F16, tag="u1")
            u1b = scr.tile([P, T], F16, tag="u1b")
            pick.tt(out, x2, c2, mult, 2 * T)          # aligned cos products
            pick.tt(u2, x2, s2sw, mult, 2 * T)         # swapped sin products
            pick.tt(out[:, 0, :], out[:, 0, :], u2[:, 1, :], add, T)
            pick.tt(out[:, 1, :], out[:, 1, :], u2[:, 0, :], add, T)
            pick.tt(u1, x1, c1[:, 0, :], mult, T)
            pick.tt(u1b, x1s, s1[:, 0, :], mult, T)
            pick.tt(out[:, 0, :], out[:, 0, :], u1, add, T)
            pick.tt(out[:, 0, :], out[:, 0, :], u1b, add, T)

        def attn(q_aps, k_aps, v_lhs, blks, is_b):
            """q_aps/k_aps: per-d-chunk [P,T] f16 APs (d on partitions).
            v_lhs(c, vc): stationary [P, d] AP for k-chunk c, out-chunk vc.
            blks: output 128-row blocks (1 for B, 2 for A)."""
            ndc = len(q_aps)
            BW = 512  # PSUM bank width in f32: matmul outs must stay in-bank

            def pieces(c):
                q0 = P * c
                return [(r, max(BW * r, q0), BW * (r + 1))
                        for r in range(T // BW) if BW * (r + 1) > max(BW * r, q0)]

            def last_c(r):
                return min(TK, (BW // P) * (r + 1)) - 1

            den = dpsum.tile([P, T], F32, tag="den")
            yt = ypsum.tile([P, T], F32, tag="yt")
            pts = []
            for c in range(TK):
                q0 = P * c
                n = T - q0
                sT = spsum.tile([P, T], F32, tag="sT")
                for (r, lo, hi) in pieces(c):
                    for dc in range(ndc):
                        nc.tensor.matmul(
                            sT[:, lo:hi], k_aps[dc][:, q0:q0 + P], q_aps[dc][:, lo:hi],
                            start=(dc == 0), stop=(dc == ndc - 1))
                pt = ptp.tile([P, n], F16, tag=f"pt{c}", name=f"pt{c}")
                pick.act_reserve(n * 0.8333 + 185).activation(pt, sT[:, q0:], EXP)
                # causal diagonal: zero exp() where q < k inside the diag block
                pick.pool_reserve(150).affine_select(
                    out=pt[:, 0:P], in_=pt[:, 0:P],
                    compare_op=mybir.AluOpType.is_ge, fill=0.0,
                    base=0, pattern=[[1, P]], channel_multiplier=-1)
                for (r, lo, hi) in pieces(c):
                    nc.tensor.matmul(yt[:, lo:hi], v_lhs(c, 0), pt[:, lo - q0:hi - q0],
                                     start=(c == 0), stop=(c == last_c(r)))
                    nc.tensor.matmul(den[:, lo:hi], ones, pt[:, lo - q0:hi - q0],
                                     start=(c == 0), stop=(c == last_c(r)))
                pts.append(pt)
            # reciprocal split per PSUM bank: the low half of den closes its
            # accumulation at c=3, so rec(low) runs while c=4..7 still fill
            # the high half -> den frees almost immediately after its last
            # matmul and the next head's den chain doesn't stall PE.
            rec = recp.tile([P, T], F16, tag="rec")
            with nc.allow_low_precision(reason="1/den fits fp16; den in [1, 8e3]"):
                for r in range(T // BW):
                    pick.dve_reserve(BW * 1.0417 + 125).reciprocal(
                        rec[:, BW * r:BW * (r + 1)], den[:, BW * r:BW * (r + 1)])
            if not is_b:
                yc0 = ycp.tile([P, T], F16, tag="yc")
                pick.psum_drain(yc0, yt, T)
                yt2 = ypsum.tile([P, T], F32, tag="yt")
                for c in range(TK):
                    q0 = P * c
                    for (r, lo, hi) in pieces(c):
                        nc.tensor.matmul(yt2[:, lo:hi], v_lhs(c, 1),
                                         pts[c][:, lo - q0:hi - q0],
                                         start=(c == 0), stop=(c == last_c(r)))
                yc1 = ycp.tile([P, T], F16, tag="yc")
                pick.psum_drain(yc1, yt2, T)
                pick.tt(outacc[:, blks[0], :], yc0, rec, mult, T)
                pick.tt(outacc[:, blks[1], :], yc1, rec, mult, T)
            else:
                tmp = scr.tile([P, T], F16, tag="btmp")
                pick.tt_psum(tmp, yt, rec, mult, T)
                pick.tt(outacc[:, blks[0], :], outacc[:, blks[0], :], tmp, add, T)
                nc.sync.dma_start(out=outT[P * blks[0]:P * (blks[0] + 1), :],
                                  in_=outacc[:, blks[0], :])

        # ================= config A =================
        for i in range(cfg.NA):
            r1, r1s = slice(P * i, P * (i + 1)), slice(P * i, P * (i + 1))
            r2 = slice(256 * i, 256 * (i + 1))
            q1 = rawp.tile([P, T], F16, tag="q1")
            nc.sync.dma_start(out=q1, in_=D["qT1"][r1, :])
            q1s = rawp.tile([P, T], F16, tag="q1s")
            nc.sync.dma_start(out=q1s, in_=D["qT1s64"][r1s, :])
            q2 = rawp.tile([P, 2, T], F16, tag="q2")
            nc.sync.dma_start(out=q2, in_=D["qT2"][r2, :].rearrange("(c p) t -> p c t", p=P))
            qmix = mixp.tile([P, 2, T], F16, tag="qmix")
            mix_A(qmix, q1, q1s, q2, tabs["tA1c"], tabs["tA1s"], tabs["tA2c"], tabs["tA2s"])

            k1 = rawp.tile([P, T], F16, tag="k1")
            nc.sync.dma_start(out=k1, in_=D["kTa1"][r1, :])
            k1s = rawp.tile([P, T], F16, tag="k1s")
            nc.sync.dma_start(out=k1s, in_=D["kTa1s64"][r1s, :])
            k2 = rawp.tile([P, 2, T], F16, tag="k2")
            nc.sync.dma_start(out=k2, in_=D["kTa2"][r2, :].rearrange("(c p) t -> p c t", p=P))
            kmix = mixp.tile([P, 2, T], F16, tag="kmix")
            mix_A(kmix, k1, k1s, k2, tabs["tA1c"], tabs["tA1s"], tabs["tA2c"], tabs["tA2s"])

            v1 = rawp.tile([P, TK, P], F16, tag="v1")
            nc.sync.dma_start(out=v1, in_=D["va1p"][r1, :].rearrange("p (c d) -> p c d", d=P))
            v2 = rawp.tile([P, TK, 2 * P], F16, tag="v2")
            nc.sync.dma_start(out=v2, in_=D["va2p"][r1, :].rearrange("p (c d) -> p c d", d=2 * P))
            vmix = mixp.tile([P, TK, 2 * P], F16, tag="vmix")
            pick.tsp_mul(vmix, v2, wv[:, 1:2], 2 * T)
            u = scr.tile([P, TK, P], F16, tag="vu")
            pick.tsp_mul(u, v1, wv[:, 0:1], T)
            pick.tt(vmix[:, :, 0:P], vmix[:, :, 0:P], u, add, T)

            attn([qmix[:, 0, :], qmix[:, 1, :]],
                 [kmix[:, 0, :], kmix[:, 1, :]],
                 lambda c, vc: vmix[:, c, P * vc:P * (vc + 1)],
                 (2 * i, 2 * i + 1), is_b=False)
            if i == 0:  # B tables: queue behind A0's loads, ready long before B
                for nm in ("tB1c", "tB1s", "tB2c", "tB2s"):
                    load_tab(nm)

        # ================= config B =================
        for j in range(cfg.NKVB):  # kv head j serves B-heads (2j, 2j+1)
            rj = slice(P * j, P * (j + 1))
            k2 = rawp.tile([P, T], F16, tag="q1")
            nc.sync.dma_start(out=k2, in_=D["kTa1"][rj, :])
            k2s = rawp.tile([P, T], F16, tag="q1s")
            nc.sync.dma_start(out=k2s, in_=D["kTa1s64"][rj, :])
            kmix = mixp.tile([P, T], F16, tag="bkmix")
            u = scr.tile([P, T], F16, tag="u1")
            pick.tt(kmix, k2, tabs["tB2c"][:, 0, :], mult, T)
            pick.tt(u, k2s, tabs["tB2s"][:, 0, :], mult, T)
            pick.tt(kmix, kmix, u, add, T)
            if j % 2 == 0:
                # packed d=64 kv pair (kv j, j+1): rows [128u0, 128u0+128)
                u0 = j // 2
                k1p = rawp.tile([P, T], F16, tag="bk1p")
                nc.sync.dma_start(out=k1p, in_=D["kTb1"][P * u0:P * (u0 + 1), :])
                k1ps = rawp.tile([P, T], F16, tag="bk1ps")
                nc.sync.dma_start(out=k1ps, in_=D["kTb1s32"][P * u0:P * (u0 + 1), :])
                tp = scr.tile([P, T], F16, tag="btp")
                tpb = scr.tile([P, T], F16, tag="btpb")
                pick.tt(tp, k1p, tabs["tB1c"][:, 0, :], mult, T)
                pick.tt(tpb, k1ps, tabs["tB1s"][:, 0, :], mult, T)
                pick.tt(tp, tp, tpb, add, T)
                last_tp = tp
                pick.tt(kmix[0:64, :], kmix[0:64, :], tp[0:64, :], add, T)
            else:
                tc2 = scr.tile([P, T], F16, tag="btc")
                pick.copy(tc2[0:64, :], last_tp[64:128, :], T)
                pick.tt(kmix[0:64, :], kmix[0:64, :], tc2[0:64, :], add, T)

            v2 = rawp.tile([P, TK, P], F16, tag="v1")
            nc.sync.dma_start(out=v2, in_=D["va1p"][rj, :].rearrange("p (c d) -> p c d", d=P))
            v1 = rawp.tile([P, TK, 64], F16, tag="bv1")
            nc.sync.dma_start(out=v1, in_=D["vb1p"][rj, :].rearrange("p (c d) -> p c d", d=64))
            vmix = mixp.tile([P, TK, P], F16, tag="bvmix")
            pick.tsp_mul(vmix, v2, wv[:, 3:4], T)
            uv = scr.tile([P, TK, 64], F16, tag="bvu")
            pick.tsp_mul(uv, v1, wv[:, 2:3], T // 2)
            pick.tt(vmix[:, :, 0:64], vmix[:, :, 0:64], uv, add, T // 2)

            # q pair for heads (2j, 2j+1)
            q2p = rawp.tile([P, 2, T], F16, tag="q2")
            nc.sync.dma_start(out=q2p, in_=D["qT2"][256 * j:256 * (j + 1), :]
                              .rearrange("(c p) t -> p c t", p=P))
            q2ps = rawp.tile([P, 2, T], F16, tag="k2")
            nc.sync.dma_start(out=q2ps, in_=D["qT2s64"][256 * j:256 * (j + 1), :]
                              .rearrange("(c p) t -> p c t", p=P))
            q1p = rawp.tile([P, T], F16, tag="k1")
            nc.sync.dma_start(out=q1p, in_=D["qT1"][rj, :])
            q1ps = rawp.tile([P, T], F16, tag="k1s")
            nc.sync.dma_start(out=q1ps, in_=D["qT1s32"][rj, :])

            qp = mixp.tile([P, 2, T], F16, tag="bqp")
            uq = scr.tile([P, 2, T], F16, tag="u2")
            for hh in range(2):
                pick.tt(qp[:, hh, :], q2p[:, hh, :], tabs["tB2c"][:, 0, :], mult, T)
                pick.tt(uq[:, hh, :], q2ps[:, hh, :], tabs["tB2s"][:, 0, :], mult, T)
            pick.tt(qp, qp, uq, add, 2 * T)
            t1 = scr.tile([P, T], F16, tag="u1")
            t1b = scr.tile([P, T], F16, tag="u1b")
            pick.tt(t1, q1p, tabs["tB1c"][:, 0, :], mult, T)
            pick.tt(t1b, q1ps, tabs["tB1s"][:, 0, :], mult, T)
            pick.tt(t1, t1, t1b, add, T)
            pick.tt(qp[0:64, 0, :], qp[0:64, 0, :], t1[0:64, :], add, T)
            tcq = scr.tile([P, T], F16, tag="btc")
            pick.copy(tcq[0:64, :], t1[64:128, :], T)
            pick.tt(qp[0:64, 1, :], qp[0:64, 1, :], tcq[0:64, :], add, T)

            for hh in range(2):
                attn([qp[:, hh, :]], [kmix],
                     lambda c, vc: vmix[:, c, :],
                     (2 * j + hh,), is_b=True)

    nc.compile()
    return nc


# ---------------------------------------------------------------------------
# Host side
# ---------------------------------------------------------------------------

def _rope_tab(pos, d, f):
    """Transposed rope tables [d, T]: (f*cos, f*sin with rot sign folded)."""
    inv = 1.0 / (10000.0 ** (np.arange(0, d, 2, dtype=np.float32) / d))
    ang = inv[:, None] * pos[None, :].astype(np.float32)      # [d/2, T]
    ang = np.concatenate([ang, ang], 0)                        # [d, T]
    c = (f * np.cos(ang)).astype(np.float32)
    s = (f * np.sin(ang)).astype(np.float32)
    s[: d // 2] *= -1.0
    return c, s


def _sig(a, half):
    """Row permutation: swap halves of size `half` in each 2*half group."""
    out = a.reshape(-1, 2, half, a.shape[-1])[:, ::-1]
    return out.reshape(a.shape)


def _vperm(vslc, dh):
    """[T, nh*dh] -> [nh, P, T//P, dh] contiguous per-partition rows."""
    T = vslc.shape[0]
    nh = vslc.shape[1] // dh
    # [c, p, head, d] -> [head, p, c, d]
    return vslc.reshape(T // P, P, nh, dh).transpose(2, 1, 0, 3)


def make_core_inputs(q, k, v, pos, weights, s, cfg: KCfg = FULL):
    """q,k,v: [T, 2048] fp32 for one batch; returns the per-core input dict."""
    f16 = lambda a: np.ascontiguousarray(a, dtype=np.float16)
    qT1 = q[:, 512 * s:512 * s + 512].T
    qT2 = q[:, 1024 * s:1024 * s + 1024].T
    kTa1 = k[:, 512 * s:512 * s + 512].T
    kTa2 = k[:, 1024 * s:1024 * s + 1024].T
    kTb1 = k[:, 256 * s:256 * s + 256].T
    arrs = {
        "qT1": f16(qT1), "qT1s64": f16(_sig(qT1, 64)), "qT1s32": f16(_sig(qT1, 32)),
        "qT2": f16(qT2), "qT2s64": f16(_sig(qT2, 64)),
        "kTa1": f16(kTa1), "kTa1s64": f16(_sig(kTa1, 64)),
        "kTa2": f16(kTa2),
        "kTb1": f16(kTb1), "kTb1s32": f16(_sig(kTb1, 32)),
        "va1p": f16(_vperm(v[:, 512 * s:512 * s + 512], 128).reshape(4 * P, -1)),
        "va2p": f16(_vperm(v[:, 1024 * s:1024 * s + 1024], 256).reshape(4 * P, -1)),
        "vb1p": f16(_vperm(v[:, 256 * s:256 * s + 256], 64).reshape(4 * P, -1)),
    }
    fA = math.sqrt(1.0 / 16.0)
    fB = math.sqrt(1.0 / math.sqrt(128.0))
    c1, s1 = _rope_tab(pos, 128, fA * float(weights[0]))
    c2, s2 = _rope_tab(pos, 256, fA * float(weights[1]))
    cb1h, sb1h = _rope_tab(pos, 64, fB * float(weights[2]))
    cb2, sb2 = _rope_tab(pos, 128, fB * float(weights[3]))
    arrs.update({
        "tA1c": f16(c1), "tA1s": f16(s1),
        # tA2s half-swapped: row block 0 holds the sin factors for x2[:,0,:]
        # (which contribute to out dim-chunk 1), block 1 those for x2[:,1,:].
        "tA2c": f16(c2), "tA2s": f16(np.vstack([s2[128:], s2[:128]])),
        "tB1c": f16(np.vstack([cb1h, cb1h])), "tB1s": f16(np.vstack([sb1h, sb1h])),
        "tB2c": f16(cb2), "tB2s": f16(sb2),
        "wvec": np.tile(np.asarray(weights, np.float32)[None, :], (P, 1)),
    })
    return arrs


_PROGRAM_CACHE = {}
TRACE = False
LAST_RESULT = None


def kernel(q_m, k_m, v_m, weights, attention_mask, position_ids):
    global LAST_RESULT
    from concourse.bass_utils import run_bass_kernel_spmd

    cfg = FULL
    q_m = np.asarray(q_m, np.float32)
    k_m = np.asarray(k_m, np.float32)
    v_m = np.asarray(v_m, np.float32)
    weights = np.asarray(weights, np.float32)
    attention_mask = np.asarray(attention_mask, np.float32)
    position_ids = np.asarray(position_ids)
    B, T, H = q_m.shape

    # the device program hardcodes the causal structure; verify it holds
    causal = np.where(np.tril(np.ones((T, T), bool)), 0.0, NEG).astype(np.float32)
    for b in range(B):
        assert np.array_equal(attention_mask[b, 0], causal), "non-causal mask"

    if "nc" not in _PROGRAM_CACHE:
        _PROGRAM_CACHE["nc"] = build_program(cfg)
    nc = _PROGRAM_CACHE["nc"]

    in_maps = []
    for b in range(B):
        for s in range(2):
            in_maps.append(make_core_inputs(
                q_m[b], k_m[b], v_m[b], position_ids[b], weights, s, cfg))
    res = run_bass_kernel_spmd(nc, in_maps, list(range(8)), trace=TRACE)
    LAST_RESULT = res
    out = np.zeros((B, T, H), np.float32)
    for b in range(B):
        for s in range(2):
            out[b, :, 1024 * s:1024 * s + 1024] = \
                res.results[2 * b + s]["outT"].astype(np.float32).T
    return out


# revision 19
# speedup vs baseline: 1.6587x; 1.0152x over previous
"""Trainium2 Bass kernel for nn_MixedAttnHeadEmbed (mixed-head-config attention).

Math (per batch b):
  Two attention configs share q_m/k_m/v_m [B,T,2048]:
    A: h=8  heads, d_max=256, mixing e in {1024,2048} -> d in {128,256}, weights w0,w1
    B: h=16 heads, d_max=128, mixing e in {1024,2048} -> d in {64,128},  weights w2,w3
  Each config: per-head q/k slices are RoPE'd, weight-summed (padded to d_max),
  GQA (8 kv heads), causal softmax attention; outputs of both configs sum.

Sharding: 8 cores = 4 batches x 2 shards. Shard s owns A-heads [4s,4s+4) and
B-heads [8s,8s+8) -> both write output columns [1024s, 1024s+1024) which are
summed on device; per-core output is the transposed block outT [1024, T] (fp16).

Device design notes (cost-model driven):
  * Everything on the elementwise path is fp16: DVE tensor_tensor gets the
    2x_1p fast mode, tensor_copy/tensor_scalar get 4x, DMA bytes halve, and
    fp16 matmuls stream at 1 cycle/row at ANY output width (fp32r pays 4x
    under 256).  Accumulation stays fp32 in PSUM; softmax input is fp32.
  * rotate_half operands arrive pre-permuted from HBM (sigma64/sigma32 row
    permutations are folded into extra DMA loads) so RoPE needs no on-chip
    shuffles or cross-partition copies: each mix is pure mult/add passes.
  * Scores are computed transposed (sT[k,q]) so softmax'd weights feed the
    y^T matmul directly; softmax is max-free (scores provably < 2), the
    denominator comes from an all-ones stationary matmul, and the causal
    diagonal is enforced by zeroing exp() outputs with affine_select (Pool,
    SBUF) instead of adding -1e9 to PSUM scores (DVE).
  * PSUM budget (8 banks): sT [P,1024]f32 double-buffered (4) + y (2) +
    den (2).  Config-A heads keep exp() outputs pt_c in SBUF and run the
    second output-d-chunk as a pure-matmul second pass over them.
  * ACT drains y PSUM tiles to SBUF fp16 copies so the single y region is
    released to the next accumulation chain at copy speed; DVE only does
    reciprocal + cheap fp16 normalize multiplies.
  * A ns-calibrated static balancer spreads mix passes across DVE/Pool
    (and ACT for copies) around the reserved exp/mask/normalize work.
"""

import math
from contextlib import ExitStack
from dataclasses import dataclass

import numpy as np

import concourse.bass as bass
import concourse.mybir as mybir
import concourse.tile as tile
from concourse import bacc

F32 = mybir.dt.float32
F16 = mybir.dt.float16
NEG = -1e9
P = 128


@dataclass(frozen=True)
class KCfg:
    T: int = 1024       # sequence length
    NA: int = 4         # config-A heads per core (d_max=256)
    NB: int = 8         # config-B heads per core (d_max=128); must be 2*NA

    @property
    def TK(self):
        return self.T // P

    @property
    def NKVB(self):
        return self.NB // 2

    @property
    def ROWS(self):
        return self.NA * 256  # == NB * 128 output rows per core


FULL = KCfg()


def _in_specs(cfg: KCfg):
    T = cfg.T
    na, nb = cfg.NA, cfg.NKVB
    return {
        # q/k transposed [cols, T]; *_s64/_s32 are rotate-half row permutations
        "qT1": (na * 128, T), "qT1s64": (na * 128, T), "qT1s32": (na * 128, T),
        "qT2": (na * 256, T), "qT2s64": (na * 256, T),
        "kTa1": (na * 128, T), "kTa1s64": (na * 128, T),
        "kTa2": (na * 256, T),
        "kTb1": (nb * 64, T), "kTb1s32": (nb * 64, T),
        # v pre-permuted per head: rows = head*P + p, cols = (chunk, d) flat
        "va1p": (na * P, (T // P) * 128),
        "va2p": (na * P, (T // P) * 256),
        "vb1p": (nb * P, (T // P) * 64),
        # rope tables (weights & score scale folded, rot sign folded in sin)
        "tA1c": (128, T), "tA1s": (128, T),
        "tA2c": (256, T), "tA2s": (256, T),   # tA2s is half-SWAPPED (see host)
        "tB1c": (128, T), "tB1s": (128, T),
        "tB2c": (128, T), "tB2s": (128, T),
        "wvec": (P, 4),
    }


class _Pick:
    """Static ns-accurate load balancer across DVE / Pool(GPSIMD) / ACT.

    v1 cost model: engine time = free_size * cycle_t (DVE 1/0.96GHz, Pool &
    ACT 1/1.2GHz) with DVE fast modes: fp16 TT 2x, fp16 sbuf copy/TSP 4x.
    Init adders: DVE +60ns sbuf / +125ns psum, ACT +185ns, Pool ~+40ns."""

    def __init__(self, nc):
        self.nc = nc
        self.load = {"dve": 0.0, "pool": 0.0, "act": 0.0}

    # --- reservations for work that must sit on one engine ---
    def act_reserve(self, ns):
        self.load["act"] += ns
        return self.nc.scalar

    def pool_reserve(self, ns):
        self.load["pool"] += ns
        return self.nc.gpsimd

    def dve_reserve(self, ns):
        self.load["dve"] += ns
        return self.nc.vector

    # --- balanced ops ---
    def tt(self, out, a, b, op, free, fast=True, psum=False):
        d = free * 1.0417 * (0.5 if (fast and not psum) else 1.0) + (125 if psum else 60)
        if psum:
            self.load["dve"] += d
            self.nc.vector.tensor_tensor(out, a, b, op)
            return
        p = free * 0.8333 + 40
        if self.load["dve"] + d <= self.load["pool"] + p:
            self.load["dve"] += d
            self.nc.vector.tensor_tensor(out, a, b, op)
        else:
            self.load["pool"] += p
            self.nc.gpsimd.tensor_tensor(out, a, b, op)

    def copy(self, dst, src, free):
        costs = {"dve": free * 0.2604 + 60, "pool": free * 0.8333 + 40,
                 "act": free * 0.8333 + 217}
        eng = min(costs, key=lambda k: self.load[k] + costs[k])
        self.load[eng] += costs[eng]
        if eng == "act":
            self.nc.scalar.copy(dst, src)
        elif eng == "pool":
            self.nc.gpsimd.tensor_copy(dst, src)
        else:
            self.nc.vector.tensor_copy(dst, src)

    def tsp_mul(self, out, in0, scalar, free):
        self.load["dve"] += free * 0.2604 + 60
        self.nc.vector.tensor_scalar_mul(out, in0, scalar)

    def psum_drain(self, dst, src, free):
        """Copy a PSUM f32 tile to SBUF fp16: DVE or ACT (no fast modes)."""
        dcost = free * 1.0417 + 125
        acost = free * 0.8333 + 217
        if self.load["dve"] + dcost <= self.load["act"] + acost:
            self.load["dve"] += dcost
            self.nc.vector.tensor_copy(dst, src)
        else:
            self.load["act"] += acost
            self.nc.scalar.copy(dst, src)

    def tt_psum(self, out, a, b, op, free):
        """2-input op with a PSUM operand: DVE only."""
        self.load["dve"] += free * 1.0417 + 125
        self.nc.vector.tensor_tensor(out, a, b, op)


def build_program(cfg: KCfg = FULL):
    nc = bacc.Bacc("TRN2", target_bir_lowering=False)
    T, TK = cfg.T, cfg.TK
    mult, add = mybir.AluOpType.mult, mybir.AluOpType.add

    D = {}
    for name, shape in _in_specs(cfg).items():
        dt = F32 if name == "wvec" else F16
        D[name] = nc.declare_dram_parameter(name, list(shape), dt, isOutput=False)
    outT = nc.declare_dram_parameter("outT", [cfg.ROWS, T], F16, isOutput=True)
    RB = cfg.ROWS // P

    with ExitStack() as ctx:
        tc = ctx.enter_context(tile.TileContext(nc))
        const = ctx.enter_context(tc.tile_pool(name="const", bufs=1))
        rawp = ctx.enter_context(tc.tile_pool(name="raw", bufs=2))
        mixp = ctx.enter_context(tc.tile_pool(name="mix", bufs=2))
        scr = ctx.enter_context(tc.tile_pool(name="scr", bufs=2))
        ptp = ctx.enter_context(tc.tile_pool(name="pt", bufs=2))
        ycp = ctx.enter_context(tc.tile_pool(name="yc", bufs=2))
        recp = ctx.enter_context(tc.tile_pool(name="rec", bufs=2))
        accp = ctx.enter_context(tc.tile_pool(name="acc", bufs=1))
        spsum = ctx.enter_context(tc.tile_pool(name="spsum", bufs=2, space="PSUM"))
        ypsum = ctx.enter_context(tc.tile_pool(name="ypsum", bufs=1, space="PSUM"))
        dpsum = ctx.enter_context(tc.tile_pool(name="dpsum", bufs=1, space="PSUM"))

        pick = _Pick(nc)

        # ---- constants ----
        ones_f = const.tile([P, P], F32, name="ones_f")
        nc.vector.memset(ones_f, 1.0)
        ones = const.tile([P, P], F16)
        nc.vector.tensor_copy(ones, ones_f)
        tabs = {}

        def load_tab(nm):
            rows = _in_specs(cfg)[nm][0]
            tl = const.tile([P, rows // P, T], F16, name=nm, tag=nm)
            tabs[nm] = tl
            nc.sync.dma_start(out=tl, in_=D[nm].rearrange("(c p) t -> p c t", p=P))

        # A-phase tables only: B tables load later, off the warmup critical path
        for nm in ("tA1c", "tA1s", "tA2c", "tA2s"):
            load_tab(nm)
        wv = const.tile([P, 4], F32)
        nc.sync.dma_start(out=wv, in_=D["wvec"][:, :])

        outacc = accp.tile([P, RB, T], F16)

        EXP = mybir.ActivationFunctionType.Exp

        def mix_A(out, x1, x1s, x2, c1, s1, c2, s2sw):
            """out [P,2,T] f16 = RoPE-mix of a config-A q or k head.
            x2 [P,2,T] (d=256), x1/x1s [P,T] (d=128, x1s sigma64-permuted).
            s2sw is the half-swapped signed d=256 sin table."""
            u2 = scr.tile([P, 2, T], F16, tag="u2")
            u1 = scr.tile([P, T], F16, tag="u1")
            u1b = scr.tile([P, T], F16, tag="u1b")
            pick.tt(out, x2, c2, mult, 2 * T)          # aligned cos products
            pick.tt(u2, x2, s2sw, mult, 2 * T)         # swapped sin products
            pick.tt(out[:, 0, :], out[:, 0, :], u2[:, 1, :], add, T)
            pick.tt(out[:, 1, :], out[:, 1, :], u2[:, 0, :], add, T)
            pick.tt(u1, x1, c1[:, 0, :], mult, T)
            pick.tt(u1b, x1s, s1[:, 0, :], mult, T)
            pick.tt(out[:, 0, :], out[:, 0, :], u1, add, T)
            pick.tt(out[:, 0, :], out[:, 0, :], u1b, add, T)

        BW = 512      # PSUM bank width in f32: matmul outs must stay in-bank
        NR = T // BW

        def pieces(c):
            q0 = P * c
            return [(r, max(BW * r, q0), BW * (r + 1))
                    for r in range(NR) if BW * (r + 1) > max(BW * r, q0)]

        def last_c(r):
            return min(TK, (BW // P) * (r + 1)) - 1

        def flush(blk):
            nc.sync.dma_start(out=outT[P * blk:P * (blk + 1), :],
                              in_=outacc[:, blk, :])

        def attn(q_aps, k_aps, v_lhs, blks, store, do_flush):
            """q_aps/k_aps: per-d-chunk [P,T] f16 APs (d on partitions).
            v_lhs(c, vc): stationary [P, d] AP for k-chunk c, out-chunk vc.
            blks: output 128-row blocks (1 for B, 2 for A).  y and den live
            in per-bank PSUM tiles so each bank's chain (low closes at c=3)
            releases its readers early despite tile-granular dep tracking."""
            ndc = len(q_aps)
            denB = [dpsum.tile([P, BW], F32, tag=f"den{r}", name=f"den{r}")
                    for r in range(NR)]
            ytB = [ypsum.tile([P, BW], F32, tag=f"yt{r}", name=f"yt{r}")
                   for r in range(NR)]
            pts = []
            for c in range(TK):
                q0 = P * c
                n = T - q0
                sT = spsum.tile([P, T], F32, tag="sT")
                for (r, lo, hi) in pieces(c):
                    for dc in range(ndc):
                        nc.tensor.matmul(
                            sT[:, lo:hi], k_aps[dc][:, q0:q0 + P], q_aps[dc][:, lo:hi],
                            start=(dc == 0), stop=(dc == ndc - 1))
                pt = ptp.tile([P, n], F16, tag=f"pt{c}", name=f"pt{c}")
                pick.act_reserve(n * 0.8333 + 185).activation(pt, sT[:, q0:], EXP)
                # causal diagonal: zero exp() where q < k inside the diag block
                pick.pool_reserve(150).affine_select(
                    out=pt[:, 0:P], in_=pt[:, 0:P],
                    compare_op=mybir.AluOpType.is_ge, fill=0.0,
                    base=0, pattern=[[1, P]], channel_multiplier=-1)
                for (r, lo, hi) in pieces(c):
                    nc.tensor.matmul(ytB[r][:, lo - BW * r:hi - BW * r],
                                     v_lhs(c, 0), pt[:, lo - q0:hi - q0],
                                     start=(c == 0), stop=(c == last_c(r)))
                    nc.tensor.matmul(denB[r][:, lo - BW * r:hi - BW * r],
                                     ones, pt[:, lo - q0:hi - q0],
                                     start=(c == 0), stop=(c == last_c(r)))
                pts.append(pt)
            rec = recp.tile([P, T], F16, tag="rec")
            with nc.allow_low_precision(reason="1/den fits fp16; den in [1, 8e3]"):
                for r in range(NR):
                    pick.dve_reserve(BW * 1.0417 + 125).reciprocal(
                        rec[:, BW * r:BW * (r + 1)], denB[r])
            if len(blks) == 2:
                yc0 = ycp.tile([P, T], F16, tag="yc")
                for r in range(NR):
                    pick.psum_drain(yc0[:, BW * r:BW * (r + 1)], ytB[r], BW)
                yt2B = [ypsum.tile([P, BW], F32, tag=f"yt{r}", name=f"yt{r}b")
                        for r in range(NR)]
                for c in range(TK):
                    q0 = P * c
                    for (r, lo, hi) in pieces(c):
                        nc.tensor.matmul(yt2B[r][:, lo - BW * r:hi - BW * r],
                                         v_lhs(c, 1), pts[c][:, lo - q0:hi - q0],
                                         start=(c == 0), stop=(c == last_c(r)))
                yc1 = ycp.tile([P, T], F16, tag="yc")
                for r in range(NR):
                    pick.psum_drain(yc1[:, BW * r:BW * (r + 1)], yt2B[r], BW)
                for bi, yc in ((0, yc0), (1, yc1)):
                    if store:
                        pick.tt(outacc[:, blks[bi], :], yc, rec, mult, T)
                    else:
                        tmp = scr.tile([P, T], F16, tag="btmp")
                        pick.tt(tmp, yc, rec, mult, T)
                        pick.tt(outacc[:, blks[bi], :], outacc[:, blks[bi], :],
                                tmp, add, T)
                    if do_flush:
                        flush(blks[bi])
            else:
                if store:
                    for r in range(NR):
                        sl = slice(BW * r, BW * (r + 1))
                        pick.tt_psum(outacc[:, blks[0], sl], ytB[r], rec[:, sl], mult, BW)
                else:
                    tmp = scr.tile([P, T], F16, tag="btmp")
                    for r in range(NR):
                        sl = slice(BW * r, BW * (r + 1))
                        pick.tt_psum(tmp[:, sl], ytB[r], rec[:, sl], mult, BW)
                    for r in range(NR):
                        sl = slice(BW * r, BW * (r + 1))
                        pick.tt(outacc[:, blks[0], sl], outacc[:, blks[0], sl],
                                tmp[:, sl], add, BW)
                if do_flush:
                    flush(blks[0])

        # ---- per-head builders --------------------------------------
        def do_A(i, store, do_flush):
            r1 = slice(P * i, P * (i + 1))
            r2 = slice(256 * i, 256 * (i + 1))
            q1 = rawp.tile([P, T], F16, tag="q1")
            nc.sync.dma_start(out=q1, in_=D["qT1"][r1, :])
            q1s = rawp.tile([P, T], F16, tag="q1s")
            nc.sync.dma_start(out=q1s, in_=D["qT1s64"][r1, :])
            q2 = rawp.tile([P, 2, T], F16, tag="q2")
            nc.sync.dma_start(out=q2, in_=D["qT2"][r2, :].rearrange("(c p) t -> p c t", p=P))
            qmix = mixp.tile([P, 2, T], F16, tag="qmix")
            mix_A(qmix, q1, q1s, q2, tabs["tA1c"], tabs["tA1s"], tabs["tA2c"], tabs["tA2s"])

            k1 = rawp.tile([P, T], F16, tag="k1")
            nc.sync.dma_start(out=k1, in_=D["kTa1"][r1, :])
            k1s = rawp.tile([P, T], F16, tag="k1s")
            nc.sync.dma_start(out=k1s, in_=D["kTa1s64"][r1, :])
            k2 = rawp.tile([P, 2, T], F16, tag="k2")
            nc.sync.dma_start(out=k2, in_=D["kTa2"][r2, :].rearrange("(c p) t -> p c t", p=P))
            kmix = mixp.tile([P, 2, T], F16, tag="kmix")
            mix_A(kmix, k1, k1s, k2, tabs["tA1c"], tabs["tA1s"], tabs["tA2c"], tabs["tA2s"])

            v1 = rawp.tile([P, TK, P], F16, tag="v1")
            nc.sync.dma_start(out=v1, in_=D["va1p"][r1, :].rearrange("p (c d) -> p c d", d=P))
            v2 = rawp.tile([P, TK, 2 * P], F16, tag="v2")
            nc.sync.dma_start(out=v2, in_=D["va2p"][r1, :].rearrange("p (c d) -> p c d", d=2 * P))
            vmix = mixp.tile([P, TK, 2 * P], F16, tag="vmix")
            pick.tsp_mul(vmix, v2, wv[:, 1:2], 2 * T)
            u = scr.tile([P, TK, P], F16, tag="vu")
            pick.tsp_mul(u, v1, wv[:, 0:1], T)
            pick.tt(vmix[:, :, 0:P], vmix[:, :, 0:P], u, add, T)

            attn([qmix[:, 0, :], qmix[:, 1, :]],
                 [kmix[:, 0, :], kmix[:, 1, :]],
                 lambda c, vc: vmix[:, c, P * vc:P * (vc + 1)],
                 (2 * i, 2 * i + 1), store=store, do_flush=do_flush)

        tp_hold = {}

        def do_B(j, store, do_flush):
            rj = slice(P * j, P * (j + 1))
            k2 = rawp.tile([P, T], F16, tag="q1")
            nc.sync.dma_start(out=k2, in_=D["kTa1"][rj, :])
            k2s = rawp.tile([P, T], F16, tag="q1s")
            nc.sync.dma_start(out=k2s, in_=D["kTa1s64"][rj, :])
            kmix = mixp.tile([P, T], F16, tag="bkmix")
            u = scr.tile([P, T], F16, tag="u1")
            pick.tt(kmix, k2, tabs["tB2c"][:, 0, :], mult, T)
            pick.tt(u, k2s, tabs["tB2s"][:, 0, :], mult, T)
            pick.tt(kmix, kmix, u, add, T)
            if j % 2 == 0:
                # packed d=64 kv pair (kv j, j+1): rows [128u0, 128u0+128)
                u0 = j // 2
                k1p = rawp.tile([P, T], F16, tag="bk1p")
                nc.sync.dma_start(out=k1p, in_=D["kTb1"][P * u0:P * (u0 + 1), :])
                k1ps = rawp.tile([P, T], F16, tag="bk1ps")
                nc.sync.dma_start(out=k1ps, in_=D["kTb1s32"][P * u0:P * (u0 + 1), :])
                tp = scr.tile([P, T], F16, tag="btp")
                tpb = scr.tile([P, T], F16, tag="btpb")
                pick.tt(tp, k1p, tabs["tB1c"][:, 0, :], mult, T)
                pick.tt(tpb, k1ps, tabs["tB1s"][:, 0, :], mult, T)
                pick.tt(tp, tp, tpb, add, T)
                tp_hold[0] = tp
                pick.tt(kmix[0:64, :], kmix[0:64, :], tp[0:64, :], add, T)
            else:
                tc2 = scr.tile([P, T], F16, tag="btc")
                pick.copy(tc2[0:64, :], tp_hold[0][64:128, :], T)
                pick.tt(kmix[0:64, :], kmix[0:64, :], tc2[0:64, :], add, T)

            v2 = rawp.tile([P, TK, P], F16, tag="v1")
            nc.sync.dma_start(out=v2, in_=D["va1p"][rj, :].rearrange("p (c d) -> p c d", d=P))
            v1 = rawp.tile([P, TK, 64], F16, tag="bv1")
            nc.sync.dma_start(out=v1, in_=D["vb1p"][rj, :].rearrange("p (c d) -> p c d", d=64))
            vmix = mixp.tile([P, TK, P], F16, tag="bvmix")
            pick.tsp_mul(vmix, v2, wv[:, 3:4], T)
            uv = scr.tile([P, TK, 64], F16, tag="bvu")
            pick.tsp_mul(uv, v1, wv[:, 2:3], T // 2)
            pick.tt(vmix[:, :, 0:64], vmix[:, :, 0:64], uv, add, T // 2)

            # q pair for heads (2j, 2j+1)
            q2p = rawp.tile([P, 2, T], F16, tag="q2")
            nc.sync.dma_start(out=q2p, in_=D["qT2"][256 * j:256 * (j + 1), :]
                              .rearrange("(c p) t -> p c t", p=P))
            q2ps = rawp.tile([P, 2, T], F16, tag="k2")
            nc.sync.dma_start(out=q2ps, in_=D["qT2s64"][256 * j:256 * (j + 1), :]
                              .rearrange("(c p) t -> p c t", p=P))
            q1p = rawp.tile([P, T], F16, tag="k1")
            nc.sync.dma_start(out=q1p, in_=D["qT1"][rj, :])
            q1ps = rawp.tile([P, T], F16, tag="k1s")
            nc.sync.dma_start(out=q1ps, in_=D["qT1s32"][rj, :])

            qp = mixp.tile([P, 2, T], F16, tag="bqp")
            uq = scr.tile([P, 2, T], F16, tag="u2")
            for hh in range(2):
                pick.tt(qp[:, hh, :], q2p[:, hh, :], tabs["tB2c"][:, 0, :], mult, T)
                pick.tt(uq[:, hh, :], q2ps[:, hh, :], tabs["tB2s"][:, 0, :], mult, T)
            pick.tt(qp, qp, uq, add, 2 * T)
            t1 = scr.tile([P, T], F16, tag="u1")
            t1b = scr.tile([P, T], F16, tag="u1b")
            pick.tt(t1, q1p, tabs["tB1c"][:, 0, :], mult, T)
            pick.tt(t1b, q1ps, tabs["tB1s"][:, 0, :], mult, T)
            pick.tt(t1, t1, t1b, add, T)
            pick.tt(qp[0:64, 0, :], qp[0:64, 0, :], t1[0:64, :], add, T)
            tcq = scr.tile([P, T], F16, tag="btc")
            pick.copy(tcq[0:64, :], t1[64:128, :], T)
            pick.tt(qp[0:64, 1, :], qp[0:64, 1, :], tcq[0:64, :], add, T)

            for hh in range(2):
                attn([qp[:, hh, :]], [kmix],
                     lambda c, vc: vmix[:, c, :],
                     (2 * j + hh,), store=store, do_flush=do_flush)

        # ---- schedule: B pair 0 first (short mix chain covers PE warmup),
        # then A heads (A0 adds onto B0's blocks), then remaining B pairs.
        for nm in ("tB1c", "tB1s", "tB2c", "tB2s"):
            load_tab(nm)
        do_B(0, store=True, do_flush=False)
        for nm in ("tA1c", "tA1s", "tA2c", "tA2s"):
            load_tab(nm)
        do_A(0, store=False, do_flush=True)
        for i in range(1, cfg.NA):
            do_A(i, store=True, do_flush=False)
        for j in range(1, cfg.NKVB):
            do_B(j, store=False, do_flush=True)

    nc.compile()
    return nc


# ---------------------------------------------------------------------------
# Host side
# ---------------------------------------------------------------------------

def _rope_tab(pos, d, f):
    """Transposed rope tables [d, T]: (f*cos, f*sin with rot sign folded)."""
    inv = 1.0 / (10000.0 ** (np.arange(0, d, 2, dtype=np.float32) / d))
    ang = inv[:, None] * pos[None, :].astype(np.float32)      # [d/2, T]
    ang = np.concatenate([ang, ang], 0)                        # [d, T]
    c = (f * np.cos(ang)).astype(np.float32)
    s = (f * np.sin(ang)).astype(np.float32)
    s[: d // 2] *= -1.0
    return c, s


def _sig(a, half):
    """Row permutation: swap halves of size `half` in each 2*half group."""
    out = a.reshape(-1, 2, half, a.shape[-1])[:, ::-1]
    return out.reshape(a.shape)


def _vperm(vslc, dh):
    """[T, nh*dh] -> [nh, P, T//P, dh] contiguous per-partition rows."""
    T = vslc.shape[0]
    nh = vslc.shape[1] // dh
    # [c, p, head, d] -> [head, p, c, d]
    return vslc.reshape(T // P, P, nh, dh).transpose(2, 1, 0, 3)


def make_core_inputs(q, k, v, pos, weights, s, cfg: KCfg = FULL):
    """q,k,v: [T, 2048] fp32 for one batch; returns the per-core input dict."""
    f16 = lambda a: np.ascontiguousarray(a, dtype=np.float16)
    qT1 = q[:, 512 * s:512 * s + 512].T
    qT2 = q[:, 1024 * s:1024 * s + 1024].T
    kTa1 = k[:, 512 * s:512 * s + 512].T
    kTa2 = k[:, 1024 * s:1024 * s + 1024].T
    kTb1 = k[:, 256 * s:256 * s + 256].T
    arrs = {
        "qT1": f16(qT1), "qT1s64": f16(_sig(qT1, 64)), "qT1s32": f16(_sig(qT1, 32)),
        "qT2": f16(qT2), "qT2s64": f16(_sig(qT2, 64)),
        "kTa1": f16(kTa1), "kTa1s64": f16(_sig(kTa1, 64)),
        "kTa2": f16(kTa2),
        "kTb1": f16(kTb1), "kTb1s32": f16(_sig(kTb1, 32)),
        "va1p": f16(_vperm(v[:, 512 * s:512 * s + 512], 128).reshape(4 * P, -1)),
        "va2p": f16(_vperm(v[:, 1024 * s:1024 * s + 1024], 256).reshape(4 * P, -1)),
        "vb1p": f16(_vperm(v[:, 256 * s:256 * s + 256], 64).reshape(4 * P, -1)),
    }
    fA = math.sqrt(1.0 / 16.0)
    fB = math.sqrt(1.0 / math.sqrt(128.0))
    c1, s1 = _rope_tab(pos, 128, fA * float(weights[0]))
    c2, s2 = _rope_tab(pos, 256, fA * float(weights[1]))
    cb1h, sb1h = _rope_tab(pos, 64, fB * float(weights[2]))
    cb2, sb2 = _rope_tab(pos, 128, fB * float(weights[3]))
    arrs.update({
        "tA1c": f16(c1), "tA1s": f16(s1),
        # tA2s half-swapped: row block 0 holds the sin factors for x2[:,0,:]
        # (which contribute to out dim-chunk 1), block 1 those for x2[:,1,:].
        "tA2c": f16(c2), "tA2s": f16(np.vstack([s2[128:], s2[:128]])),
        "tB1c": f16(np.vstack([cb1h, cb1h])), "tB1s": f16(np.vstack([sb1h, sb1h])),
        "tB2c": f16(cb2), "tB2s": f16(sb2),
        "wvec": np.tile(np.asarray(weights, np.float32)[None, :], (P, 1)),
    })
    return arrs


_PROGRAM_CACHE = {}
TRACE = False
LAST_RESULT = None


def kernel(q_m, k_m, v_m, weights, attention_mask, position_ids):
    global LAST_RESULT
    from concourse.bass_utils import run_bass_kernel_spmd

    cfg = FULL
    q_m = np.asarray(q_m, np.float32)
    k_m = np.asarray(k_m, np.float32)
    v_m = np.asarray(v_m, np.float32)
    weights = np.asarray(weights, np.float32)
    attention_mask = np.asarray(attention_mask, np.float32)
    position_ids = np.asarray(position_ids)
    B, T, H = q_m.shape

    # the device program hardcodes the causal structure; verify it holds
    causal = np.where(np.tril(np.ones((T, T), bool)), 0.0, NEG).astype(np.float32)
    for b in range(B):
        assert np.array_equal(attention_mask[b, 0], causal), "non-causal mask"

    if "nc" not in _PROGRAM_CACHE:
        _PROGRAM_CACHE["nc"] = build_program(cfg)
    nc = _PROGRAM_CACHE["nc"]

    in_maps = []
    for b in range(B):
        for s in range(2):
            in_maps.append(make_core_inputs(
                q_m[b], k_m[b], v_m[b], position_ids[b], weights, s, cfg))
    res = run_bass_kernel_spmd(nc, in_maps, list(range(8)), trace=TRACE)
    LAST_RESULT = res
    out = np.zeros((B, T, H), np.float32)
    for b in range(B):
        for s in range(2):
            out[b, :, 1024 * s:1024 * s + 1024] = \
                res.results[2 * b + s]["outT"].astype(np.float32).T
    return out
